# revision 2
# baseline (speedup 1.0000x reference)
"""GATConv x2 + MLP head GNN over 8 Trainium2 cores — cost-model-tuned v2.

Structure (4 SPMD launches; host does only index manipulation / gathers):
  l0: per-node u = x@W1 (fp16), es1/ed1 attention terms.
  l1: GAT layer 1 over fp16 slot stream [u0 u1 u2 es1]; per-tile
      tensor_scalar tb, big stt lrelu, big ACT exp, fused stt product+accum
      for M (split DVE/gpsimd), ts+accum for s.  Tail: x2, es4, ed4.
  l2: GAT layer 2 over fp16 stream [x2(3) es4(4)] (same shape as l1 but
      4 heads); tail: h2, PE transpose -> h2T, fc matmul (bias via aug
      ones row), BN partial stats. Outputs h3T bf16 + stats.
  l3: BN finalize/apply, two bf16 matmuls + sigmoid, outT [6, R_pad].
Slot streams are packed per chunk of tiles: [128, npl, CH] fp16 with
per-tile column ranges; pad slots carry es=-3e38 (exp->0), dummy rows get
one live zero slot so s>0.
"""

import numpy as np
import ml_dtypes

import concourse.bass as bass
import concourse.bacc as bacc
import concourse.tile as tile
from concourse import mybir
from concourse.bass_utils import run_bass_kernel_spmd
from concourse.masks import make_identity

FP = mybir.dt.float32
F16 = mybir.dt.float16
BF = mybir.dt.bfloat16
AF = mybir.ActivationFunctionType
OP = mybir.AluOpType

N_CORES = 8
SLOPE = 0.2
BN_EPS = 1e-5
NEG_BIG = -60000.0  # finite in fp16; exp() still underflows to exactly 0
import os as _os
CH_MAX = int(_os.environ.get('CHM', '768'))
POOL_PROD_L1 = int(_os.environ.get('PPL1', '2'))
POOL_PROD_L2 = int(_os.environ.get('PPL2', '8'))

_PROG_CACHE = {}
LAST_RESULTS = []
LAUNCH_WALL = []


# --------------------------------------------------------------------------
# Host-side preprocessing (index manipulation only)
# --------------------------------------------------------------------------

def _preprocess(edge_index, n):
    src = np.asarray(edge_index[0], dtype=np.int64)
    dst = np.asarray(edge_index[1], dtype=np.int64)
    loops = np.arange(n, dtype=np.int64)
    src = np.concatenate([src, loops])
    dst = np.concatenate([dst, loops])

    assert n % N_CORES == 0
    R = n // N_CORES
    T = -(-R // 128)
    R_pad = T * 128

    owner = dst // R
    per_core = []
    degs = []
    for c in range(N_CORES):
        m = owner == c
        s_c = src[m]
        d_loc = dst[m] - c * R
        deg = np.bincount(d_loc, minlength=R)
        row_of = np.argsort(-deg, kind="stable")
        per_core.append((s_c, d_loc, deg[row_of], row_of))
        degs.append(deg[row_of])

    tile_k = np.zeros(T, dtype=np.int64)
    for t in range(T):
        lo, hi = t * 128, min(t * 128 + 128, R)
        kmax = 1
        if hi > lo:
            for c in range(N_CORES):
                kmax = max(kmax, int(degs[c][lo:hi].max()))
        tile_k[t] = -(-max(kmax, 1) // 4) * 4
    tile_off = np.concatenate([[0], np.cumsum(tile_k * 128)])
    S = int(tile_off[-1])

    # chunks of tiles with total column budget (ramped up at the start
    # so the first compute can begin after a small DMA)
    chunks = []
    t0 = 0
    budgets = [CH_MAX // 4, CH_MAX // 2]
    while t0 < T:
        bud = budgets[len(chunks)] if len(chunks) < len(budgets) else CH_MAX
        t1, ch = t0, 0
        while t1 < T and (t1 == t0 or ch + tile_k[t1] <= bud):
            ch += tile_k[t1]
            t1 += 1
        chunks.append((t0, t1, int(ch)))
        t0 = t1

    # slot -> source node (or -1 pad / -2 dummy-live), canonical numbering
    # pos = tile_off[t] + p*K_t + j
    slots_all = np.full((N_CORES, S), -1, dtype=np.int64)
    rows_node = np.empty((N_CORES, R), dtype=np.int64)
    for c in range(N_CORES):
        s_c, d_loc, deg_sorted, row_of = per_core[c]
        rank_of = np.empty(R, dtype=np.int64)
        rank_of[row_of] = np.arange(R)
        rows_node[c] = row_of + c * R

        erow = rank_of[d_loc]
        eorder = np.argsort(erow, kind="stable")
        erow_s = erow[eorder]
        esrc_s = s_c[eorder]
        row_start = np.concatenate([[0], np.cumsum(deg_sorted)])
        slot_no = np.arange(len(erow_s)) - row_start[erow_s]
        et = erow_s // 128
        ep = erow_s % 128
        pos = tile_off[et] + ep * tile_k[et] + slot_no
        slots_all[c][pos] = esrc_s
    # dummy rows (rank >= R): slot 0 is a live zero-contribution slot
    for r in range(R, R_pad):
        t = r // 128
        p = r % 128
        slots_all[:, tile_off[t] + p * tile_k[t]] = -2

    return dict(
        R=R, R_pad=R_pad, T=T, S=S,
        tile_k=[int(k) for k in tile_k],
        tile_off=[int(o) for o in tile_off],
        chunks=chunks, slots=slots_all, rows_node=rows_node,
    )


def _perm(meta, npl):
    """stream position -> flat index into [S, npl] (slot-major gather)."""
    key = ("perm", npl)
    p = meta.get(key)
    if p is not None:
        return p
    tile_k = meta["tile_k"]
    tile_off = meta["tile_off"]
    parts = []
    p_i = np.arange(128)[:, None, None]
    c_i = np.arange(npl)[None, :, None]
    for (t0, t1, CH) in meta["chunks"]:
        cols = []
        for t in range(t0, t1):
            K = tile_k[t]
            k_i = np.arange(K)[None, None, :]
            cols.append((tile_off[t] + p_i * K + k_i) * npl + c_i)
        parts.append(np.concatenate(cols, axis=2).reshape(-1))
    p = np.concatenate(parts)
    meta[key] = p
    return p


def _make_stream(meta, pernode, es_pernode):
    """Per-core fp16 slot stream, chunk-major [128, npl, CH] layout."""
    F = pernode.shape[1]
    H = es_pernode.shape[1]
    n = pernode.shape[0]
    npl = F + H
    perm = _perm(meta, npl)
    ptab = np.vstack([pernode.astype(np.float32),
                      np.zeros((2, F), np.float32)])
    etab = np.vstack([es_pernode.astype(np.float32),
                      np.full((1, H), NEG_BIG, np.float32),
                      np.zeros((1, H), np.float32)])
    out = np.empty((N_CORES, meta["S"] * npl), dtype=np.float16)
    for c in range(N_CORES):
        sl = meta["slots"][c].copy()
        sl[sl == -1] = n
        sl[sl == -2] = n + 1
        both = np.concatenate([ptab[sl], etab[sl]], axis=1)
        out[c] = both.reshape(-1)[perm].astype(np.float16)
    return out


# --------------------------------------------------------------------------
# Device program helpers
# --------------------------------------------------------------------------

def _ld(nc, pool, dram, shape, name, dt=FP):
    t = pool.tile(shape, dt, name=name)
    nc.sync.dma_start(out=t[:], in_=dram.ap())
    return t


def _vpair(nc, pool, wb, ab, nch, nheads, name):
    """v[f,h] = sum_c W[f, h*nch+c] * a[h*nch+c] -> [128, nch, nheads]."""
    vt = pool.tile([128, nch, nheads], FP, name=name)
    for h in range(nheads):
        for cc in range(nch):
            o = vt[:, :, h:h + 1].rearrange("p f h -> p (f h)")
            wcols = wb[:].rearrange("p (f hc) -> p f hc", f=nch)[
                :, :, h * nch + cc]
            sc = ab[:, h * nch + cc:h * nch + cc + 1]
            if cc == 0:
                nc.vector.tensor_scalar(out=o, in0=wcols, scalar1=sc,
                                        scalar2=None, op0=OP.mult)
            else:
                nc.vector.scalar_tensor_tensor(out=o, in0=wcols, scalar=sc,
                                               in1=o, op0=OP.mult, op1=OP.add)
    return vt


def _rows_affine(nc, pool, T, xr, vt, nch, nheads, name, dt=FP):
    """out[p, t, h] = sum_f xr[p, t, f] * vt[f, h]."""
    out = pool.tile([128, T, nheads], dt, name=name)
    for h in range(nheads):
        o = out[:, :, h:h + 1].rearrange("p t h -> p (t h)")
        for f in range(nch):
            srcv = xr[:, :, f:f + 1].rearrange("p t f -> p (t f)")
            sc = vt[:, f, h:h + 1].rearrange("p h -> p h")
            if f == 0:
                nc.vector.tensor_scalar(out=o, in0=srcv, scalar1=sc,
                                        scalar2=None, op0=OP.mult)
            else:
                nc.vector.scalar_tensor_tensor(out=o, in0=srcv, scalar=sc,
                                               in1=o, op0=OP.mult, op1=OP.add)
    return out


def _edge_phase(nc, tc, pool, meta, g_dram, ed_rows, nch, nheads, pool_prod,
                chunk_done_cb=None, M=None, S_=None):
    """Edge aggregation over the packed fp16 chunk stream.

    g_dram: [S*npl] fp16, npl = nch + nheads, planes [x.. | es..].
    ed_rows: SBUF [128, T, nheads] fp32 per-row dst logits.
    pool_prod: planes hf >= nheads*nch - pool_prod compute their e*x product
    on the gpsimd engine (plain tensor_tensor), with a cheap fp16
    tensor_scalar+accum on DVE; the rest use a fused DVE stt+accum.
    Returns (M [128, T, nheads*nch] fp32, S_ [128, T, nheads] fp32).
    """
    T = meta["T"]
    tile_k = meta["tile_k"]
    npl = nch + nheads
    if M is None:
        M = pool.tile([128, T, nheads * nch], FP, name="Macc")
    if S_ is None:
        S_ = pool.tile([128, T, nheads], FP, name="Sacc")
    scd = pool.tile([128, CH_MAX], F16, name="scd")

    npm = nheads * nch
    prb = None
    if pool_prod:
        prb = [pool.tile([128, pool_prod, CH_MAX], F16, name=f"prb{i}")
               for i in range(2)]

    def emit_accums(ci, t0, t1):
        """DVE ts+accum over the gpsimd products of chunk ci."""
        co = 0
        for t in range(t0, t1):
            K = tile_k[t]
            for j in range(pool_prod):
                hf = npm - pool_prod + j
                nc.vector.tensor_scalar(
                    out=scd[:, co:co + K],
                    in0=prb[ci % 2][:, j, co:co + K],
                    scalar1=1.0, scalar2=0.0, op0=OP.mult,
                    op1=OP.add, accum_out=M[:, t, hf:hf + 1])
            co += K

    with tc.tile_pool(name="edge", bufs=3) as ep, \
         tc.tile_pool(name="edge2", bufs=2) as e2:
        deferred = None
        for ci, (t0, t1, CH) in enumerate(meta["chunks"]):
            off0 = meta["tile_off"][t0] * npl
            off1 = meta["tile_off"][t1] * npl
            G = ep.tile([128, npl, CH], F16, tag="G")
            nc.sync.dma_start(
                out=G[:].rearrange("p c x -> p (c x)"),
                in_=g_dram.ap()[off0:off1].rearrange("(p x) -> p x", p=128))
            tb = e2.tile([128, nheads, CH], F16, tag="tb")
            lr = e2.tile([128, nheads, CH], F16, tag="lr")
            e = e2.tile([128, nheads, CH], BF, tag="e")

            tmid = (t0 + t1 + 1) // 2
            com = sum(tile_k[t] for t in range(t0, tmid))
            halves = [(t0, tmid, 0, com), (tmid, t1, com, CH)]

            def emit_tb(ta, tz, ca):
                co = ca
                for t in range(ta, tz):
                    K = tile_k[t]
                    for h in range(nheads):
                        nc.vector.tensor_scalar(
                            out=tb[:, h, co:co + K],
                            in0=G[:, nch + h, co:co + K],
                            scalar1=ed_rows[:, t, h:h + 1],
                            scalar2=None, op0=OP.add)
                    co += K

            def emit_act(ca, cz):
                nc.scalar.activation(
                    out=lr[:, :, ca:cz], in_=tb[:, :, ca:cz],
                    func=AF.Prelu, alpha=SLOPE)
                nc.scalar.activation(
                    out=e[:, :, ca:cz], in_=lr[:, :, ca:cz],
                    func=AF.Exp)

            def emit_prod(ta, tz, ca):
                co = ca
                for t in range(ta, tz):
                    K = tile_k[t]
                    for j in range(pool_prod):
                        hf = npm - pool_prod + j
                        h, f = hf // nch, hf % nch
                        nc.gpsimd.tensor_tensor(
                            out=prb[ci % 2][:, j, co:co + K],
                            in0=e[:, h, co:co + K],
                            in1=G[:, f, co:co + K], op=OP.mult)
                    co += K

            def emit_stts(ta, tz, ca):
                co = ca
                for t in range(ta, tz):
                    K = tile_k[t]
                    for h in range(nheads):
                        for f in range(nch):
                            hf = h * nch + f
                            if hf < npm - pool_prod:
                                nc.vector.scalar_tensor_tensor(
                                    out=scd[:, co:co + K],
                                    in0=e[:, h, co:co + K],
                                    scalar=1.0, in1=G[:, f, co:co + K],
                                    op0=OP.bypass, op1=OP.mult,
                                    accum_out=M[:, t, hf:hf + 1])
                        nc.vector.tensor_scalar(
                            out=scd[:, co:co + K], in0=e[:, h, co:co + K],
                            scalar1=1.0, scalar2=0.0, op0=OP.mult,
                            op1=OP.add, accum_out=S_[:, t, h:h + 1])
                    co += K

            (ta0, tz0, ca0, cz0), (ta1, tz1, ca1, cz1) = halves
            emit_tb(ta0, tz0, ca0)
            emit_act(ca0, cz0)         # ACT half A starts
            emit_tb(ta1, tz1, ca1)     # DVE does half-B tb meanwhile
            if pool_prod and deferred is not None:
                emit_accums(*deferred)  # ready DVE work during ACT latency
            if cz1 > ca1:
                emit_act(ca1, cz1)
            emit_prod(ta0, tz0, ca0)
            emit_stts(ta0, tz0, ca0)
            emit_prod(ta1, tz1, ca1)
            emit_stts(ta1, tz1, ca1)
            if chunk_done_cb is not None:
                done_hi = meta["chunks"][ci - 1][1] if pool_prod else t1
                if deferred is None and pool_prod:
                    done_hi = 0
                chunk_done_cb(done_hi)
            deferred = (ci, t0, t1)
        if pool_prod and deferred is not None:
            emit_accums(*deferred)
        if chunk_done_cb is not None:
            chunk_done_cb(meta["T"])
    return M, S_


def _gat_head_out(nc, pool, T, M, S_, wb, bb, nch, nheads, name, dt=F16):
    """out[:, t, hc] = relu((sum_f M[h,f] W[f,hc]) / s_h + b[hc]).

    wb None => stream carried pre-projected features (M_hf is already the
    per-output-channel aggregate): out_hc = relu(M_hc / s_h + b[hc])."""
    rn = pool.tile([128, T, nheads], FP, name=name + "rn")
    nc.vector.reciprocal(rn[:].rearrange("p t h -> p (t h)"),
                         S_[:].rearrange("p t h -> p (t h)"))
    out = pool.tile([128, T, nheads * nch], dt, name=name)
    t1 = pool.tile([128, T], FP, name=name + "t1")
    t2 = pool.tile([128, T], FP, name=name + "t2")
    for h in range(nheads):
        for c in range(nch):
            hc = h * nch + c
            if wb is None:
                t1v = M[:, :, hc:hc + 1].rearrange("p t c -> p (t c)")
            else:
                for f in range(nch):
                    srcv = M[:, :, h * nch + f:h * nch + f + 1].rearrange(
                        "p t c -> p (t c)")
                    wsc = wb[:].rearrange("p (f hc) -> p f hc", f=nch)[
                        :, f, hc:hc + 1]
                    if f == 0:
                        nc.vector.tensor_scalar(
                            out=t1[:], in0=srcv, scalar1=wsc,
                            scalar2=None, op0=OP.mult)
                    else:
                        nc.vector.scalar_tensor_tensor(
                            out=t1[:], in0=srcv, scalar=wsc, in1=t1[:],
                            op0=OP.mult, op1=OP.add)
                t1v = t1[:]
            nc.vector.tensor_tensor(
                out=t2[:], in0=t1v,
                in1=rn[:, :, h:h + 1].rearrange("p t h -> p (t h)"),
                op=OP.mult)
            nc.vector.tensor_scalar(
                out=out[:, :, hc:hc + 1].rearrange("p t c -> p (t c)"),
                in0=t2[:], scalar1=bb[:, hc:hc + 1], scalar2=0.0,
                op0=OP.add, op1=OP.max)
    return out


# --------------------------------------------------------------------------
# Launch builders
# --------------------------------------------------------------------------

def _build_l0(meta):
    nc = bacc.Bacc("TRN2", target_bir_lowering=False, debug=False,
                   num_devices=N_CORES)
    T = meta["T"]
    B = 3 * T
    xr_i = nc.dram_tensor("xr", [128, B + 15], FP, kind="ExternalInput")
    uo = nc.dram_tensor("u", [128, 4 * T], F16, kind="ExternalOutput")
    ed1o = nc.dram_tensor("ed1", [128, T], FP, kind="ExternalOutput")

    with tile.TileContext(nc) as tc:
        with tc.tile_pool(name="p", bufs=1) as pool:
            allin = pool.tile([128, B + 15], FP, name="allin")
            nc.sync.dma_start(out=allin[:], in_=xr_i.ap())
            u = pool.tile([128, 4, T], F16, name="u")
            ed1 = pool.tile([128, T], FP, name="ed1")
            # u[:, f, :] = sum_c x_c * W1[c, f]
            for f in range(3):
                o = u[:, f, :]
                for c in range(3):
                    srcv = allin[:, c * T:(c + 1) * T]
                    w0 = B + c * 3 + f
                    if c == 0:
                        nc.vector.tensor_scalar(
                            out=o, in0=srcv, scalar1=allin[:, w0:w0 + 1],
                            scalar2=None, op0=OP.mult)
                    else:
                        nc.vector.scalar_tensor_tensor(
                            out=o, in0=srcv, scalar=allin[:, w0:w0 + 1],
                            in1=o, op0=OP.mult, op1=OP.add)
            # es1 (u plane 3) and ed1: dot of u planes with a_src1/a_dst1
            for f in range(3):
                a0 = B + 9 + f
                d0 = B + 12 + f
                if f == 0:
                    nc.vector.tensor_scalar(
                        out=u[:, 3, :], in0=u[:, 0, :],
                        scalar1=allin[:, a0:a0 + 1], scalar2=None,
                        op0=OP.mult)
                    nc.vector.tensor_scalar(
                        out=ed1[:], in0=u[:, 0, :],
                        scalar1=allin[:, d0:d0 + 1], scalar2=None,
                        op0=OP.mult)
                else:
                    nc.vector.scalar_tensor_tensor(
                        out=u[:, 3, :], in0=u[:, f, :],
                        scalar=allin[:, a0:a0 + 1], in1=u[:, 3, :],
                        op0=OP.mult, op1=OP.add)
                    nc.vector.scalar_tensor_tensor(
                        out=ed1[:], in0=u[:, f, :],
                        scalar=allin[:, d0:d0 + 1], in1=ed1[:],
                        op0=OP.mult, op1=OP.add)
            nc.sync.dma_start(out=uo.ap(),
                              in_=u[:].rearrange("p c t -> p (c t)"))
            nc.sync.dma_start(out=ed1o.ap(), in_=ed1[:])
    nc.compile()
    return nc


def _build_l1(meta):
    nc = bacc.Bacc("TRN2", target_bir_lowering=False, debug=False,
                   num_devices=N_CORES)
    T, S, R_pad = meta["T"], meta["S"], meta["R_pad"]
    g1 = nc.dram_tensor("g1", [S * 4], F16, kind="ExternalInput")
    ed1i = nc.dram_tensor("ed1", [128, T], FP, kind="ExternalInput")
    b1b = nc.dram_tensor("b1b", [128, 3], FP, kind="ExternalInput")
    w2b = nc.dram_tensor("w2b", [128, 36], FP, kind="ExternalInput")
    as2 = nc.dram_tensor("as2", [128, 12], FP, kind="ExternalInput")
    ad2 = nc.dram_tensor("ad2", [128, 12], FP, kind="ExternalInput")
    x2o = nc.dram_tensor("x2", [128, T * 3], F16, kind="ExternalOutput")
    es4o = nc.dram_tensor("es4", [128, T * 4], F16, kind="ExternalOutput")
    ed4o = nc.dram_tensor("ed4", [128, T * 4], FP, kind="ExternalOutput")

    with tile.TileContext(nc) as tc:
        with tc.tile_pool(name="p", bufs=1) as pool:
            b_b = _ld(nc, pool, b1b, [128, 3], "b_b")
            w2 = _ld(nc, pool, w2b, [128, 36], "w2")
            a_s2 = _ld(nc, pool, as2, [128, 12], "a_s2")
            a_d2 = _ld(nc, pool, ad2, [128, 12], "a_d2")
            ed = pool.tile([128, T, 1], FP, name="ed")
            nc.sync.dma_start(out=ed[:].rearrange("p t c -> p (t c)"),
                              in_=ed1i.ap())
            M, S_ = _edge_phase(nc, tc, pool, meta, g1, ed, 3, 1,
                                pool_prod=POOL_PROD_L1)
            x2 = _gat_head_out(nc, pool, T, M, S_, None, b_b, 3, 1, "x2",
                               dt=F16)
            vs2 = _vpair(nc, pool, w2, a_s2, 3, 4, "vs2")
            vd2 = _vpair(nc, pool, w2, a_d2, 3, 4, "vd2")
            es4 = _rows_affine(nc, pool, T, x2, vs2, 3, 4, "es4r", dt=F16)
            ed4 = _rows_affine(nc, pool, T, x2, vd2, 3, 4, "ed4r")
            nc.sync.dma_start(out=x2o.ap(),
                              in_=x2[:].rearrange("p t c -> p (t c)"))
            nc.sync.dma_start(out=es4o.ap(),
                              in_=es4[:].rearrange("p t c -> p (t c)"))
            nc.sync.dma_start(out=ed4o.ap(),
                              in_=ed4[:].rearrange("p t c -> p (t c)"))
    nc.compile()
    return nc


def _build_l2(meta, n):
    nc = bacc.Bacc("TRN2", target_bir_lowering=False, debug=False,
                   num_devices=N_CORES)
    T, S, R_pad, R = meta["T"], meta["S"], meta["R_pad"], meta["R"]
    g2 = nc.dram_tensor("g2", [S * 7], F16, kind="ExternalInput")
    ed4i = nc.dram_tensor("ed4", [128, T * 4], FP, kind="ExternalInput")
    w2blk = nc.dram_tensor("w2blk", [12, 12], F16, kind="ExternalInput")
    b2c = nc.dram_tensor("b2c", [12, 1], FP, kind="ExternalInput")
    fcwa = nc.dram_tensor("fcwa", [13, 128], F16, kind="ExternalInput")
    h3o = nc.dram_tensor("h3", [128, R_pad], F16, kind="ExternalOutput")
    sto = nc.dram_tensor("bnstat", [128, 2], FP, kind="ExternalOutput")

    GRP = 24
    with tile.TileContext(nc) as tc:
        with tc.tile_pool(name="p", bufs=1) as pool, \
             tc.tile_pool(name="tps", bufs=4, space="PSUM") as tps, \
             tc.tile_pool(name="mms", bufs=2, space="PSUM") as mms, \
             tc.tile_pool(name="fps", bufs=2, space="PSUM") as fps:
            w2b_s = _ld(nc, pool, w2blk, [12, 12], "w2b_s", dt=F16)
            b2_s = _ld(nc, pool, b2c, [12, 1], "b2_s")
            fcw = _ld(nc, pool, fcwa, [13, 128], "fcw", dt=F16)
            ed = pool.tile([128, T, 4], FP, name="ed")
            nc.scalar.dma_start(out=ed[:].rearrange("p t c -> p (t c)"),
                                in_=ed4i.ap())
            ident = pool.tile([128, 128], F16, name="ident")
            make_identity(nc, ident[:])
            rn = pool.tile([128, T, 4], FP, name="rn")
            Mn = pool.tile([128, T, 12], F16, name="Mn")
            MnT = pool.tile([12, R_pad], F16, name="MnT")
            h2T = pool.tile([13, R_pad], F16, name="h2T")
            nc.gpsimd.memset(h2T[:], 1.0)
            h3T = pool.tile([128, R_pad], F16, name="h3T")
            bns = pool.tile([128, 32], FP, name="bns")
            sqa = pool.tile([128, 32], FP, name="sqa")
            nc.vector.memset(bns[:], 0.0)
            nc.vector.memset(sqa[:], 0.0)
            sqs = pool.tile([128, 512], F16, name="sqs")

            M = pool.tile([128, T, 12], FP, name="Macc")
            S_ = pool.tile([128, T, 4], FP, name="Sacc")
            st = {"norm": 0, "tp": 0, "h2": 0, "fc": 0, "dma": 0}

            def tail_cb(hi):
                # 1. reciprocal + normalize for finished tiles
                ta, tz = st["norm"], hi
                if tz > ta:
                    nc.vector.reciprocal(
                        rn[:, ta:tz, :].rearrange("p t h -> p (t h)"),
                        S_[:, ta:tz, :].rearrange("p t h -> p (t h)"))
                    for hf in range(12):
                        h = hf // 3
                        nc.vector.tensor_tensor(
                            out=Mn[:, ta:tz, hf:hf + 1].rearrange(
                                "p t c -> p (t c)"),
                            in0=M[:, ta:tz, hf:hf + 1].rearrange(
                                "p t c -> p (t c)"),
                            in1=rn[:, ta:tz, h:h + 1].rearrange(
                                "p t c -> p (t c)"), op=OP.mult)
                    st["norm"] = tz
                # 2. transposes in 4-tile batches
                while st["tp"] + 4 <= st["norm"] or \
                        (st["norm"] >= T and st["tp"] < T):
                    g0 = st["tp"]
                    g1_ = min(g0 + 4, T)
                    ps = tps.tile([12, 512], F16, tag="tp")
                    for j, t in enumerate(range(g0, g1_)):
                        nc.tensor.transpose(
                            out=ps[:, j * 128:(j + 1) * 128],
                            in_=Mn[:, t, :], identity=ident[:])
                    nc.scalar.activation(
                        out=MnT[0:12, g0 * 128:g1_ * 128],
                        in_=ps[:, 0:(g1_ - g0) * 128], func=AF.Copy)
                    st["tp"] = g1_
                # 3. h2 matmul in 512-col chunks
                tcols = st["tp"] * 128
                while st["h2"] + 512 <= tcols or \
                        (st["tp"] >= T and st["h2"] < R_pad):
                    j0 = st["h2"]
                    j1 = min(j0 + 512, tcols)
                    ps2 = mms.tile([12, j1 - j0], FP, tag="h2m")
                    nc.tensor.matmul(ps2[:], lhsT=w2b_s[:],
                                     rhs=MnT[:, j0:j1], start=True,
                                     stop=True)
                    nc.scalar.activation(out=h2T[0:12, j0:j1], in_=ps2[:],
                                         func=AF.Relu, bias=b2_s[:, 0:1],
                                         scale=1.0)
                    st["h2"] = j1
                if st["h2"] >= R_pad and R_pad > R and not st.get("zr"):
                    nc.gpsimd.memset(h2T[:, R:R_pad], 0.0)
                    st["zr"] = True
                # 4. fc in 512-col chunks; the chunk containing the dummy
                # columns waits until they are zeroed
                lastchunk = (R // 512) * 512
                fcmax = min(st["h2"], lastchunk) if not st.get("zr") \
                    else (st["h2"] if st["h2"] < R_pad else R_pad)
                while st["fc"] + 512 <= fcmax or \
                        (st["h2"] >= R_pad and st["fc"] < R_pad):
                    j0 = st["fc"]
                    j1 = min(j0 + 512, R_pad)
                    psf = fps.tile([128, j1 - j0], FP, tag="fc")
                    nc.tensor.matmul(psf[:], lhsT=fcw[:], rhs=h2T[:, j0:j1],
                                     start=True, stop=True)
                    i = j0 // 512
                    nc.scalar.activation(out=h3T[:, j0:j1], in_=psf[:],
                                         func=AF.Relu,
                                         accum_out=bns[:, i:i + 1])
                    nc.scalar.activation(out=sqs[:, 0:j1 - j0],
                                         in_=h3T[:, j0:j1], func=AF.Square,
                                         accum_out=sqa[:, i:i + 1])
                    st["fc"] = j1
                # 5. h3 write-out in 2048-col pieces
                while st["dma"] + 2048 <= st["fc"] or \
                        (st["fc"] >= R_pad and st["dma"] < R_pad):
                    j0 = st["dma"]
                    j1 = min(j0 + 2048, R_pad)
                    nc.sync.dma_start(out=h3o.ap()[:, j0:j1],
                                      in_=h3T[:, j0:j1])
                    st["dma"] = j1

            _edge_phase(nc, tc, pool, meta, g2, ed, 3, 4,
                        pool_prod=POOL_PROD_L2, chunk_done_cb=tail_cb,
                        M=M, S_=S_)

            st2 = pool.tile([128, 2], FP, name="st2")
            nc.vector.tensor_reduce(out=st2[:, 0:1], in_=bns[:],
                                    axis=mybir.AxisListType.X, op=OP.add)
            nc.vector.tensor_reduce(out=st2[:, 1:2], in_=sqa[:],
                                    axis=mybir.AxisListType.X, op=OP.add)
            nc.sync.dma_start(out=sto.ap(), in_=st2[:])
    nc.compile()
    return nc


def _build_l3(meta, n):
    nc = bacc.Bacc("TRN2", target_bir_lowering=False, debug=False,
                   num_devices=N_CORES)
    R_pad = meta["R_pad"]
    h3i = nc.dram_tensor("h3", [128, R_pad], F16, kind="ExternalInput")
    sti = nc.dram_tensor("bnstats", [128, 16], FP, kind="ExternalInput")
    bng = nc.dram_tensor("bng", [128, 1], FP, kind="ExternalInput")
    bnb = nc.dram_tensor("bnb", [128, 1], FP, kind="ExternalInput")
    l2wa = nc.dram_tensor("l2wa", [128, 64], F16, kind="ExternalInput")
    l2bb = nc.dram_tensor("l2bb", [64, 1], FP, kind="ExternalInput")
    owa = nc.dram_tensor("owa", [65, 6], F16, kind="ExternalInput")
    outo = nc.dram_tensor("out", [6, R_pad], FP, kind="ExternalOutput")

    with tile.TileContext(nc) as tc:
        with tc.tile_pool(name="p", bufs=1) as pool, \
             tc.tile_pool(name="ps", bufs=4, space="PSUM") as pp:
            sts = _ld(nc, pool, sti, [128, 16], "sts")
            h3s = pool.tile([128, R_pad], F16, name="h3s")
            q = R_pad // 8
            for j in range(0, R_pad, q):
                nc.scalar.dma_start(out=h3s[:, j:j + q],
                                    in_=h3i.ap()[:, j:j + q])
            bng_s = _ld(nc, pool, bng, [128, 1], "bng_s")
            bnb_s = _ld(nc, pool, bnb, [128, 1], "bnb_s")
            l2w = _ld(nc, pool, l2wa, [128, 64], "l2w", dt=F16)
            l2b = _ld(nc, pool, l2bb, [64, 1], "l2b")
            ow = _ld(nc, pool, owa, [65, 6], "ow", dt=F16)

            red = pool.tile([128, 2], FP, name="red")
            nc.vector.tensor_reduce(
                out=red[:], in_=sts[:].rearrange("p (s c) -> p s c", s=2),
                axis=mybir.AxisListType.X, op=OP.add)
            mu = pool.tile([128, 1], FP, name="mu")
            nc.vector.tensor_scalar(out=mu[:], in0=red[:, 0:1],
                                    scalar1=1.0 / n, scalar2=None,
                                    op0=OP.mult)
            m2 = pool.tile([128, 1], FP, name="m2")
            nc.vector.tensor_scalar(out=m2[:], in0=red[:, 1:2],
                                    scalar1=1.0 / n, scalar2=None,
                                    op0=OP.mult)
            var = pool.tile([128, 1], FP, name="var")
            nc.vector.tensor_tensor(out=var[:], in0=mu[:], in1=mu[:],
                                    op=OP.mult)
            nc.vector.tensor_tensor(out=var[:], in0=m2[:], in1=var[:],
                                    op=OP.subtract)
            epsb = pool.tile([128, 1], FP, name="epsb")
            nc.vector.memset(epsb[:], BN_EPS)
            sd = pool.tile([128, 1], FP, name="sd")
            nc.scalar.activation(out=sd[:], in_=var[:], func=AF.Sqrt,
                                 bias=epsb[:], scale=1.0)
            rsig = pool.tile([128, 1], FP, name="rsig")
            nc.vector.reciprocal(rsig[:], sd[:])
            scale = pool.tile([128, 1], FP, name="scale")
            nc.vector.tensor_tensor(out=scale[:], in0=bng_s[:], in1=rsig[:],
                                    op=OP.mult)
            shift = pool.tile([128, 1], FP, name="shift")
            nc.vector.tensor_tensor(out=shift[:], in0=mu[:], in1=scale[:],
                                    op=OP.mult)
            nc.vector.tensor_tensor(out=shift[:], in0=bnb_s[:], in1=shift[:],
                                    op=OP.subtract)

            hbn = pool.tile([128, R_pad], F16, name="hbn")
            h4a = pool.tile([65, R_pad], F16, name="h4a")
            nc.gpsimd.memset(h4a[:], 1.0)
            outT = pool.tile([6, R_pad], FP, name="outT")
            chunks = [(j, min(j + 512, R_pad)) for j in range(0, R_pad, 512)]
            for (j0, j1) in chunks:
                nc.vector.tensor_scalar(out=hbn[:, j0:j1], in0=h3s[:, j0:j1],
                                        scalar1=scale[:], scalar2=shift[:],
                                        op0=OP.mult, op1=OP.add)
                ps = pp.tile([64, j1 - j0], FP, tag="l2")
                nc.tensor.matmul(ps[:], lhsT=l2w[:], rhs=hbn[:, j0:j1],
                                 start=True, stop=True)
                nc.vector.tensor_scalar(out=h4a[0:64, j0:j1], in0=ps[:],
                                        scalar1=l2b[:, 0:1], scalar2=None,
                                        op0=OP.add)
                ps2 = pp.tile([6, j1 - j0], FP, tag="out")
                nc.tensor.matmul(ps2[:], lhsT=ow[:], rhs=h4a[:, j0:j1],
                                 start=True, stop=True)
                nc.scalar.activation(out=outT[:, j0:j1], in_=ps2[:],
                                     func=AF.Sigmoid)
            nc.sync.dma_start(out=outo.ap(), in_=outT[:])
    nc.compile()
    return nc


# --------------------------------------------------------------------------
# Orchestration
# --------------------------------------------------------------------------

def _bcast(a, cols):
    return np.ascontiguousarray(np.broadcast_to(
        np.asarray(a, np.float32).reshape(1, -1), (128, cols)))


def _run(nc, in_maps):
    import time as _t
    t0 = _t.perf_counter()
    res = run_bass_kernel_spmd(nc, in_maps, list(range(N_CORES)))
    LAUNCH_WALL.append(_t.perf_counter() - t0)
    LAST_RESULTS.append(res)
    return res.results


def _rows_to_pernode(meta, arrs):
    R = meta["R"]
    F = arrs[0].shape[1]
    out = np.empty((R * N_CORES, F), arrs[0].dtype)
    for c in range(N_CORES):
        out[meta["rows_node"][c]] = arrs[c][:R]
    return out


def EXTRA_TSIM_BUILDERS(meta, n):
    return {
        "l0": lambda: _build_l0(meta),
        "l1": lambda: _build_l1(meta),
        "l2": lambda: _build_l2(meta, n),
        "l3": lambda: _build_l3(meta, n),
    }


def kernel(x, edge_index, W1, a_src1, a_dst1, b1, W2, a_src2, a_dst2, b2,
           fc_W, fc_b, bn_g, bn_b, l2_W, l2_b, out_W, out_b):
    global LAST_RESULTS
    LAST_RESULTS = []
    x = np.asarray(x, np.float32)
    n = x.shape[0]
    ekey = (n, np.asarray(edge_index).shape[1])
    meta = _PROG_CACHE.get(("meta", ekey))
    fp = np.asarray(edge_index)[:, :: max(1, ekey[1] // 64)]
    if meta is None or not np.array_equal(meta["_fp"], fp):
        meta = _preprocess(np.asarray(edge_index), n)
        meta["_fp"] = fp.copy()
        _PROG_CACHE.clear()
        _PROG_CACHE[("meta", ekey)] = meta
    R, R_pad = meta["R"], meta["R_pad"]
    if ("l0", ekey) not in _PROG_CACHE:
        _PROG_CACHE[("l0", ekey)] = _build_l0(meta)
        _PROG_CACHE[("l1", ekey)] = _build_l1(meta)
        _PROG_CACHE[("l2", ekey)] = _build_l2(meta, n)
        _PROG_CACHE[("l3", ekey)] = _build_l3(meta, n)

    # ---- launch 0: per-node u / es1 / ed1
    T = meta["T"]

    def to_dev(a):      # [R_pad, C] rank-major -> [128, T*C]
        C = a.shape[1]
        return np.ascontiguousarray(
            a.reshape(T, 128, C).transpose(1, 0, 2).reshape(128, T * C))

    def from_dev(a, C):  # [128, T*C] -> [R_pad, C] rank-major
        return a.reshape(128, T, C).transpose(1, 0, 2).reshape(R_pad, C)

    tail15 = np.concatenate([
        np.asarray(W1, np.float32).reshape(-1),
        np.asarray(a_src1, np.float32).reshape(-1),
        np.asarray(a_dst1, np.float32).reshape(-1)]).reshape(1, 15)
    in_maps = []
    for c in range(N_CORES):
        xr = np.zeros((R_pad, 3), np.float32)
        xr[:R] = x[meta["rows_node"][c]]
        # c-major planes [x0(T) x1(T) x2(T)] + [W1|a_src1|a_dst1]
        xrd = xr.reshape(T, 128, 3).transpose(1, 2, 0).reshape(128, 3 * T)
        in_maps.append(dict(
            xr=np.ascontiguousarray(np.concatenate(
                [xrd, np.broadcast_to(tail15, (128, 15))], axis=1))))
    r0 = _run(_PROG_CACHE[("l0", ekey)], in_maps)

    def u_unpack(a):   # [128, 4T] c-major -> [R_pad, 4] rank-major
        return a.reshape(128, 4, T).transpose(2, 0, 1).reshape(R_pad, 4)

    u4 = [u_unpack(r0[c]["u"]) for c in range(N_CORES)]
    u_pn = _rows_to_pernode(meta, [a[:, 0:3] for a in u4])
    es1_pn = _rows_to_pernode(meta, [a[:, 3:4] for a in u4])

    # ---- launch 1
    g1 = _make_stream(meta, u_pn, es1_pn)
    in_maps = []
    for c in range(N_CORES):
        in_maps.append(dict(
            g1=g1[c], ed1=np.ascontiguousarray(r0[c]["ed1"]),
            b1b=_bcast(b1, 3), w2b=_bcast(W2, 36), as2=_bcast(a_src2, 12),
            ad2=_bcast(a_dst2, 12)))
    r1 = _run(_PROG_CACHE[("l1", ekey)], in_maps)

    x2_pn = _rows_to_pernode(meta, [from_dev(r1[c]["x2"], 3)
                                    for c in range(N_CORES)])
    es4_pn = _rows_to_pernode(meta, [from_dev(r1[c]["es4"], 4)
                                    for c in range(N_CORES)])

    # ---- launch 2 (GAT2 + fc + BN partial stats)
    g2 = _make_stream(meta, x2_pn, es4_pn)
    fcwa = np.vstack([np.asarray(fc_W, np.float32),
                      np.asarray(fc_b, np.float32)[None, :]]).astype(
        np.float16)
    # block-diagonal per-head W2: w2blk[(h,f), hc] = W2[f, hc] iff hc//3 == h
    W2f = np.asarray(W2, np.float32)
    w2blk = np.zeros((12, 12), np.float32)
    for h in range(4):
        w2blk[h * 3:(h + 1) * 3, h * 3:(h + 1) * 3] = W2f[:, h * 3:(h + 1) * 3]
    w2blk = w2blk.astype(np.float16)
    in_maps = []
    for c in range(N_CORES):
        in_maps.append(dict(
            g2=g2[c], ed4=np.ascontiguousarray(r1[c]["ed4"]),
            w2blk=w2blk, b2c=np.asarray(b2, np.float32).reshape(12, 1),
            fcwa=fcwa))
    r2 = _run(_PROG_CACHE[("l2", ekey)], in_maps)

    stats = np.zeros((128, 16), np.float32)
    for c in range(N_CORES):
        stats[:, c] = r2[c]["bnstat"][:, 0]
        stats[:, 8 + c] = r2[c]["bnstat"][:, 1]

    # ---- launch 3 (BN finalize/apply + MLP output)
    l2wa = np.asarray(l2_W, np.float32).astype(np.float16)
    owa = np.vstack([np.asarray(out_W, np.float32),
                     np.asarray(out_b, np.float32)[None, :]]).astype(
        np.float16)
    in_maps = []
    for c in range(N_CORES):
        in_maps.append(dict(
            h3=r2[c]["h3"], bnstats=stats,
            bng=np.asarray(bn_g, np.float32).reshape(128, 1),
            bnb=np.asarray(bn_b, np.float32).reshape(128, 1),
            l2wa=l2wa,
            l2bb=np.asarray(l2_b, np.float32).reshape(64, 1),
            owa=owa))
    r3 = _run(_PROG_CACHE[("l3", ekey)], in_maps)

    out = np.zeros((n, 6), np.float32)
    for c in range(N_CORES):
        out[meta["rows_node"][c]] = r3[c]["out"][:, :R].T
    return out


# revision 3
# speedup vs baseline: 1.0056x; 1.0056x over previous
"""GATConv x2 + MLP head GNN over 8 Trainium2 cores — cost-model-tuned v2.

Structure (4 SPMD launches; host does only index manipulation / gathers):
  l0: per-node u = x@W1 (fp16), es1/ed1 attention terms.
  l1: GAT layer 1 over fp16 slot stream [u0 u1 u2 es1]; per-tile
      tensor_scalar tb, big stt lrelu, big ACT exp, fused stt product+accum
      for M (split DVE/gpsimd), ts+accum for s.  Tail: x2, es4, ed4.
  l2: GAT layer 2 over fp16 stream [x2(3) es4(4)] (same shape as l1 but
      4 heads); tail: h2, PE transpose -> h2T, fc matmul (bias via aug
      ones row), BN partial stats. Outputs h3T bf16 + stats.
  l3: BN finalize/apply, two bf16 matmuls + sigmoid, outT [6, R_pad].
Slot streams are packed per chunk of tiles: [128, npl, CH] fp16 with
per-tile column ranges; pad slots carry es=-3e38 (exp->0), dummy rows get
one live zero slot so s>0.
"""

import numpy as np
import ml_dtypes

import concourse.bass as bass
import concourse.bacc as bacc
import concourse.tile as tile
from concourse import mybir
from concourse.bass_utils import run_bass_kernel_spmd
from concourse.masks import make_identity

FP = mybir.dt.float32
F16 = mybir.dt.float16
BF = mybir.dt.bfloat16
AF = mybir.ActivationFunctionType
OP = mybir.AluOpType

N_CORES = 8
SLOPE = 0.2
BN_EPS = 1e-5
NEG_BIG = -60000.0  # finite in fp16; exp() still underflows to exactly 0
import os as _os
CH_MAX = int(_os.environ.get('CHM', '768'))
POOL_PROD_L1 = int(_os.environ.get('PPL1', '2'))
POOL_PROD_L2 = int(_os.environ.get('PPL2', '6'))
TB_POOL = int(_os.environ.get('TBP', '2'))

_PROG_CACHE = {}
LAST_RESULTS = []
LAUNCH_WALL = []


# --------------------------------------------------------------------------
# Host-side preprocessing (index manipulation only)
# --------------------------------------------------------------------------

def _preprocess(edge_index, n):
    src = np.asarray(edge_index[0], dtype=np.int64)
    dst = np.asarray(edge_index[1], dtype=np.int64)
    loops = np.arange(n, dtype=np.int64)
    src = np.concatenate([src, loops])
    dst = np.concatenate([dst, loops])

    assert n % N_CORES == 0
    R = n // N_CORES
    T = -(-R // 128)
    R_pad = T * 128

    owner = dst // R
    per_core = []
    degs = []
    for c in range(N_CORES):
        m = owner == c
        s_c = src[m]
        d_loc = dst[m] - c * R
        deg = np.bincount(d_loc, minlength=R)
        row_of = np.argsort(-deg, kind="stable")
        per_core.append((s_c, d_loc, deg[row_of], row_of))
        degs.append(deg[row_of])

    tile_k = np.zeros(T, dtype=np.int64)
    for t in range(T):
        lo, hi = t * 128, min(t * 128 + 128, R)
        kmax = 1
        if hi > lo:
            for c in range(N_CORES):
                kmax = max(kmax, int(degs[c][lo:hi].max()))
        tile_k[t] = -(-max(kmax, 1) // 4) * 4
    tile_off = np.concatenate([[0], np.cumsum(tile_k * 128)])
    S = int(tile_off[-1])

    # chunks of tiles with total column budget (ramped up at the start
    # so the first compute can begin after a small DMA)
    chunks = []
    t0 = 0
    budgets = [CH_MAX // 4, CH_MAX // 2]
    while t0 < T:
        bud = budgets[len(chunks)] if len(chunks) < len(budgets) else CH_MAX
        t1, ch = t0, 0
        while t1 < T and (t1 == t0 or ch + tile_k[t1] <= bud):
            ch += tile_k[t1]
            t1 += 1
        chunks.append((t0, t1, int(ch)))
        t0 = t1

    # slot -> source node (or -1 pad / -2 dummy-live), canonical numbering
    # pos = tile_off[t] + p*K_t + j
    slots_all = np.full((N_CORES, S), -1, dtype=np.int64)
    rows_node = np.empty((N_CORES, R), dtype=np.int64)
    for c in range(N_CORES):
        s_c, d_loc, deg_sorted, row_of = per_core[c]
        rank_of = np.empty(R, dtype=np.int64)
        rank_of[row_of] = np.arange(R)
        rows_node[c] = row_of + c * R

        erow = rank_of[d_loc]
        eorder = np.argsort(erow, kind="stable")
        erow_s = erow[eorder]
        esrc_s = s_c[eorder]
        row_start = np.concatenate([[0], np.cumsum(deg_sorted)])
        slot_no = np.arange(len(erow_s)) - row_start[erow_s]
        et = erow_s // 128
        ep = erow_s % 128
        pos = tile_off[et] + ep * tile_k[et] + slot_no
        slots_all[c][pos] = esrc_s
    # dummy rows (rank >= R): slot 0 is a live zero-contribution slot
    for r in range(R, R_pad):
        t = r // 128
        p = r % 128
        slots_all[:, tile_off[t] + p * tile_k[t]] = -2

    return dict(
        R=R, R_pad=R_pad, T=T, S=S,
        tile_k=[int(k) for k in tile_k],
        tile_off=[int(o) for o in tile_off],
        chunks=chunks, slots=slots_all, rows_node=rows_node,
    )


def _perm(meta, npl):
    """stream position -> flat index into [S, npl] (slot-major gather)."""
    key = ("perm", npl)
    p = meta.get(key)
    if p is not None:
        return p
    tile_k = meta["tile_k"]
    tile_off = meta["tile_off"]
    parts = []
    p_i = np.arange(128)[:, None, None]
    c_i = np.arange(npl)[None, :, None]
    for (t0, t1, CH) in meta["chunks"]:
        cols = []
        for t in range(t0, t1):
            K = tile_k[t]
            k_i = np.arange(K)[None, None, :]
            cols.append((tile_off[t] + p_i * K + k_i) * npl + c_i)
        parts.append(np.concatenate(cols, axis=2).reshape(-1))
    p = np.concatenate(parts)
    meta[key] = p
    return p


def _make_stream(meta, pernode, es_pernode):
    """Per-core fp16 slot stream, chunk-major [128, npl, CH] layout."""
    F = pernode.shape[1]
    H = es_pernode.shape[1]
    n = pernode.shape[0]
    npl = F + H
    perm = _perm(meta, npl)
    ptab = np.vstack([pernode.astype(np.float32),
                      np.zeros((2, F), np.float32)])
    etab = np.vstack([es_pernode.astype(np.float32),
                      np.full((1, H), NEG_BIG, np.float32),
                      np.zeros((1, H), np.float32)])
    out = np.empty((N_CORES, meta["S"] * npl), dtype=np.float16)
    for c in range(N_CORES):
        sl = meta["slots"][c].copy()
        sl[sl == -1] = n
        sl[sl == -2] = n + 1
        both = np.concatenate([ptab[sl], etab[sl]], axis=1)
        out[c] = both.reshape(-1)[perm].astype(np.float16)
    return out


# --------------------------------------------------------------------------
# Device program helpers
# --------------------------------------------------------------------------

def _ld(nc, pool, dram, shape, name, dt=FP):
    t = pool.tile(shape, dt, name=name)
    nc.sync.dma_start(out=t[:], in_=dram.ap())
    return t


def _vpair(nc, pool, wb, ab, nch, nheads, name):
    """v[f,h] = sum_c W[f, h*nch+c] * a[h*nch+c] -> [128, nch, nheads]."""
    vt = pool.tile([128, nch, nheads], FP, name=name)
    for h in range(nheads):
        for cc in range(nch):
            o = vt[:, :, h:h + 1].rearrange("p f h -> p (f h)")
            wcols = wb[:].rearrange("p (f hc) -> p f hc", f=nch)[
                :, :, h * nch + cc]
            sc = ab[:, h * nch + cc:h * nch + cc + 1]
            if cc == 0:
                nc.vector.tensor_scalar(out=o, in0=wcols, scalar1=sc,
                                        scalar2=None, op0=OP.mult)
            else:
                nc.vector.scalar_tensor_tensor(out=o, in0=wcols, scalar=sc,
                                               in1=o, op0=OP.mult, op1=OP.add)
    return vt


def _rows_affine(nc, pool, T, xr, vt, nch, nheads, name, dt=FP):
    """out[p, t, h] = sum_f xr[p, t, f] * vt[f, h]."""
    out = pool.tile([128, T, nheads], dt, name=name)
    for h in range(nheads):
        o = out[:, :, h:h + 1].rearrange("p t h -> p (t h)")
        for f in range(nch):
            srcv = xr[:, :, f:f + 1].rearrange("p t f -> p (t f)")
            sc = vt[:, f, h:h + 1].rearrange("p h -> p h")
            if f == 0:
                nc.vector.tensor_scalar(out=o, in0=srcv, scalar1=sc,
                                        scalar2=None, op0=OP.mult)
            else:
                nc.vector.scalar_tensor_tensor(out=o, in0=srcv, scalar=sc,
                                               in1=o, op0=OP.mult, op1=OP.add)
    return out


def _edge_phase(nc, tc, pool, meta, g_dram, ed_rows, nch, nheads, pool_prod,
                chunk_done_cb=None, M=None, S_=None, tb_pool=0):
    """Edge aggregation over the packed fp16 chunk stream.

    g_dram: [S*npl] fp16, npl = nch + nheads, planes [x.. | es..].
    ed_rows: SBUF [128, T, nheads] fp32 per-row dst logits.
    pool_prod: planes hf >= nheads*nch - pool_prod compute their e*x product
    on the gpsimd engine (plain tensor_tensor), with a cheap fp16
    tensor_scalar+accum on DVE; the rest use a fused DVE stt+accum.
    Returns (M [128, T, nheads*nch] fp32, S_ [128, T, nheads] fp32).
    """
    T = meta["T"]
    tile_k = meta["tile_k"]
    npl = nch + nheads
    if M is None:
        M = pool.tile([128, T, nheads * nch], FP, name="Macc")
    if S_ is None:
        S_ = pool.tile([128, T, nheads], FP, name="Sacc")
    scd = pool.tile([128, CH_MAX], F16, name="scd")

    npm = nheads * nch
    prb = None
    if pool_prod:
        prb = [pool.tile([128, pool_prod, CH_MAX], F16, name=f"prb{i}")
               for i in range(2)]

    def emit_accums(ci, t0, t1):
        """DVE ts+accum over the gpsimd products of chunk ci."""
        co = 0
        for t in range(t0, t1):
            K = tile_k[t]
            for j in range(pool_prod):
                hf = npm - pool_prod + j
                nc.vector.tensor_scalar(
                    out=scd[:, co:co + K],
                    in0=prb[ci % 2][:, j, co:co + K],
                    scalar1=1.0, scalar2=0.0, op0=OP.mult,
                    op1=OP.add, accum_out=M[:, t, hf:hf + 1])
            co += K

    with tc.tile_pool(name="edge", bufs=3) as ep, \
         tc.tile_pool(name="edge2", bufs=2) as e2:
        deferred = None
        for ci, (t0, t1, CH) in enumerate(meta["chunks"]):
            off0 = meta["tile_off"][t0] * npl
            off1 = meta["tile_off"][t1] * npl
            G = ep.tile([128, npl, CH], F16, tag="G")
            nc.sync.dma_start(
                out=G[:].rearrange("p c x -> p (c x)"),
                in_=g_dram.ap()[off0:off1].rearrange("(p x) -> p x", p=128))
            tb = e2.tile([128, nheads, CH], F16, tag="tb")
            lr = e2.tile([128, nheads, CH], F16, tag="lr")
            e = e2.tile([128, nheads, CH], BF, tag="e")

            tmid = (t0 + t1 + 1) // 2
            com = sum(tile_k[t] for t in range(t0, tmid))
            halves = [(t0, tmid, 0, com), (tmid, t1, com, CH)]

            def emit_tb(ta, tz, ca):
                co = ca
                for t in range(ta, tz):
                    K = tile_k[t]
                    for h in range(nheads):
                        eng = nc.gpsimd if h < tb_pool else nc.vector
                        eng.tensor_scalar(
                            out=tb[:, h, co:co + K],
                            in0=G[:, nch + h, co:co + K],
                            scalar1=ed_rows[:, t, h:h + 1],
                            scalar2=None, op0=OP.add)
                    co += K

            def emit_act(ca, cz):
                nc.scalar.activation(
                    out=lr[:, :, ca:cz], in_=tb[:, :, ca:cz],
                    func=AF.Prelu, alpha=SLOPE)
                nc.scalar.activation(
                    out=e[:, :, ca:cz], in_=lr[:, :, ca:cz],
                    func=AF.Exp)

            def emit_prod(ta, tz, ca):
                co = ca
                for t in range(ta, tz):
                    K = tile_k[t]
                    for j in range(pool_prod):
                        hf = npm - pool_prod + j
                        h, f = hf // nch, hf % nch
                        nc.gpsimd.tensor_tensor(
                            out=prb[ci % 2][:, j, co:co + K],
                            in0=e[:, h, co:co + K],
                            in1=G[:, f, co:co + K], op=OP.mult)
                    co += K

            def emit_stts(ta, tz, ca):
                co = ca
                for t in range(ta, tz):
                    K = tile_k[t]
                    for h in range(nheads):
                        for f in range(nch):
                            hf = h * nch + f
                            if hf < npm - pool_prod:
                                nc.vector.scalar_tensor_tensor(
                                    out=scd[:, co:co + K],
                                    in0=e[:, h, co:co + K],
                                    scalar=1.0, in1=G[:, f, co:co + K],
                                    op0=OP.bypass, op1=OP.mult,
                                    accum_out=M[:, t, hf:hf + 1])
                        nc.vector.tensor_scalar(
                            out=scd[:, co:co + K], in0=e[:, h, co:co + K],
                            scalar1=1.0, scalar2=0.0, op0=OP.mult,
                            op1=OP.add, accum_out=S_[:, t, h:h + 1])
                    co += K

            (ta0, tz0, ca0, cz0), (ta1, tz1, ca1, cz1) = halves
            emit_tb(ta0, tz0, ca0)
            emit_act(ca0, cz0)         # ACT half A starts
            emit_tb(ta1, tz1, ca1)     # DVE does half-B tb meanwhile
            if pool_prod and deferred is not None:
                emit_accums(*deferred)  # ready DVE work during ACT latency
            if cz1 > ca1:
                emit_act(ca1, cz1)
            emit_prod(ta0, tz0, ca0)
            emit_stts(ta0, tz0, ca0)
            emit_prod(ta1, tz1, ca1)
            emit_stts(ta1, tz1, ca1)
            if chunk_done_cb is not None:
                done_hi = meta["chunks"][ci - 1][1] if pool_prod else t1
                if deferred is None and pool_prod:
                    done_hi = 0
                chunk_done_cb(done_hi)
            deferred = (ci, t0, t1)
        if pool_prod and deferred is not None:
            emit_accums(*deferred)
        if chunk_done_cb is not None:
            chunk_done_cb(meta["T"])
    return M, S_


def _gat_head_out(nc, pool, T, M, S_, wb, bb, nch, nheads, name, dt=F16):
    """out[:, t, hc] = relu((sum_f M[h,f] W[f,hc]) / s_h + b[hc]).

    wb None => stream carried pre-projected features (M_hf is already the
    per-output-channel aggregate): out_hc = relu(M_hc / s_h + b[hc])."""
    rn = pool.tile([128, T, nheads], FP, name=name + "rn")
    nc.vector.reciprocal(rn[:].rearrange("p t h -> p (t h)"),
                         S_[:].rearrange("p t h -> p (t h)"))
    out = pool.tile([128, T, nheads * nch], dt, name=name)
    t1 = pool.tile([128, T], FP, name=name + "t1")
    t2 = pool.tile([128, T], FP, name=name + "t2")
    for h in range(nheads):
        for c in range(nch):
            hc = h * nch + c
            if wb is None:
                t1v = M[:, :, hc:hc + 1].rearrange("p t c -> p (t c)")
            else:
                for f in range(nch):
                    srcv = M[:, :, h * nch + f:h * nch + f + 1].rearrange(
                        "p t c -> p (t c)")
                    wsc = wb[:].rearrange("p (f hc) -> p f hc", f=nch)[
                        :, f, hc:hc + 1]
                    if f == 0:
                        nc.vector.tensor_scalar(
                            out=t1[:], in0=srcv, scalar1=wsc,
                            scalar2=None, op0=OP.mult)
                    else:
                        nc.vector.scalar_tensor_tensor(
                            out=t1[:], in0=srcv, scalar=wsc, in1=t1[:],
                            op0=OP.mult, op1=OP.add)
                t1v = t1[:]
            nc.vector.tensor_tensor(
                out=t2[:], in0=t1v,
                in1=rn[:, :, h:h + 1].rearrange("p t h -> p (t h)"),
                op=OP.mult)
            nc.vector.tensor_scalar(
                out=out[:, :, hc:hc + 1].rearrange("p t c -> p (t c)"),
                in0=t2[:], scalar1=bb[:, hc:hc + 1], scalar2=0.0,
                op0=OP.add, op1=OP.max)
    return out


# --------------------------------------------------------------------------
# Launch builders
# --------------------------------------------------------------------------

def _build_l0(meta):
    nc = bacc.Bacc("TRN2", target_bir_lowering=False, debug=False,
                   num_devices=N_CORES)
    T = meta["T"]
    B = 3 * T
    xr_i = nc.dram_tensor("xr", [128, B + 15], FP, kind="ExternalInput")
    uo = nc.dram_tensor("u", [128, 4 * T], F16, kind="ExternalOutput")
    ed1o = nc.dram_tensor("ed1", [128, T], FP, kind="ExternalOutput")

    with tile.TileContext(nc) as tc:
        with tc.tile_pool(name="p", bufs=1) as pool:
            allin = pool.tile([128, B + 15], FP, name="allin")
            nc.sync.dma_start(out=allin[:], in_=xr_i.ap())
            u = pool.tile([128, 4, T], F16, name="u")
            ed1 = pool.tile([128, T], FP, name="ed1")
            # u[:, f, :] = sum_c x_c * W1[c, f]
            for f in range(3):
                o = u[:, f, :]
                for c in range(3):
                    srcv = allin[:, c * T:(c + 1) * T]
                    w0 = B + c * 3 + f
                    if c == 0:
                        nc.vector.tensor_scalar(
                            out=o, in0=srcv, scalar1=allin[:, w0:w0 + 1],
                            scalar2=None, op0=OP.mult)
                    else:
                        nc.vector.scalar_tensor_tensor(
                            out=o, in0=srcv, scalar=allin[:, w0:w0 + 1],
                            in1=o, op0=OP.mult, op1=OP.add)
            # es1 (u plane 3) and ed1: dot of u planes with a_src1/a_dst1
            for f in range(3):
                a0 = B + 9 + f
                d0 = B + 12 + f
                if f == 0:
                    nc.vector.tensor_scalar(
                        out=u[:, 3, :], in0=u[:, 0, :],
                        scalar1=allin[:, a0:a0 + 1], scalar2=None,
                        op0=OP.mult)
                    nc.vector.tensor_scalar(
                        out=ed1[:], in0=u[:, 0, :],
                        scalar1=allin[:, d0:d0 + 1], scalar2=None,
                        op0=OP.mult)
                else:
                    nc.vector.scalar_tensor_tensor(
                        out=u[:, 3, :], in0=u[:, f, :],
                        scalar=allin[:, a0:a0 + 1], in1=u[:, 3, :],
                        op0=OP.mult, op1=OP.add)
                    nc.vector.scalar_tensor_tensor(
                        out=ed1[:], in0=u[:, f, :],
                        scalar=allin[:, d0:d0 + 1], in1=ed1[:],
                        op0=OP.mult, op1=OP.add)
            nc.sync.dma_start(out=uo.ap(),
                              in_=u[:].rearrange("p c t -> p (c t)"))
            nc.sync.dma_start(out=ed1o.ap(), in_=ed1[:])
    nc.compile()
    return nc


def _build_l1(meta):
    nc = bacc.Bacc("TRN2", target_bir_lowering=False, debug=False,
                   num_devices=N_CORES)
    T, S, R_pad = meta["T"], meta["S"], meta["R_pad"]
    g1 = nc.dram_tensor("g1", [S * 4], F16, kind="ExternalInput")
    ed1i = nc.dram_tensor("ed1", [128, T], FP, kind="ExternalInput")
    b1b = nc.dram_tensor("b1b", [128, 3], FP, kind="ExternalInput")
    w2b = nc.dram_tensor("w2b", [128, 36], FP, kind="ExternalInput")
    as2 = nc.dram_tensor("as2", [128, 12], FP, kind="ExternalInput")
    ad2 = nc.dram_tensor("ad2", [128, 12], FP, kind="ExternalInput")
    x2o = nc.dram_tensor("x2", [128, T * 3], F16, kind="ExternalOutput")
    es4o = nc.dram_tensor("es4", [128, T * 4], F16, kind="ExternalOutput")
    ed4o = nc.dram_tensor("ed4", [128, T * 4], FP, kind="ExternalOutput")

    with tile.TileContext(nc) as tc:
        with tc.tile_pool(name="p", bufs=1) as pool:
            b_b = _ld(nc, pool, b1b, [128, 3], "b_b")
            w2 = _ld(nc, pool, w2b, [128, 36], "w2")
            a_s2 = _ld(nc, pool, as2, [128, 12], "a_s2")
            a_d2 = _ld(nc, pool, ad2, [128, 12], "a_d2")
            ed = pool.tile([128, T, 1], FP, name="ed")
            nc.sync.dma_start(out=ed[:].rearrange("p t c -> p (t c)"),
                              in_=ed1i.ap())
            M, S_ = _edge_phase(nc, tc, pool, meta, g1, ed, 3, 1,
                                pool_prod=POOL_PROD_L1)
            x2 = _gat_head_out(nc, pool, T, M, S_, None, b_b, 3, 1, "x2",
                               dt=F16)
            vs2 = _vpair(nc, pool, w2, a_s2, 3, 4, "vs2")
            vd2 = _vpair(nc, pool, w2, a_d2, 3, 4, "vd2")
            es4 = _rows_affine(nc, pool, T, x2, vs2, 3, 4, "es4r", dt=F16)
            ed4 = _rows_affine(nc, pool, T, x2, vd2, 3, 4, "ed4r")
            nc.sync.dma_start(out=x2o.ap(),
                              in_=x2[:].rearrange("p t c -> p (t c)"))
            nc.sync.dma_start(out=es4o.ap(),
                              in_=es4[:].rearrange("p t c -> p (t c)"))
            nc.sync.dma_start(out=ed4o.ap(),
                              in_=ed4[:].rearrange("p t c -> p (t c)"))
    nc.compile()
    return nc


def _build_l2(meta, n):
    nc = bacc.Bacc("TRN2", target_bir_lowering=False, debug=False,
                   num_devices=N_CORES)
    T, S, R_pad, R = meta["T"], meta["S"], meta["R_pad"], meta["R"]
    g2 = nc.dram_tensor("g2", [S * 7], F16, kind="ExternalInput")
    ed4i = nc.dram_tensor("ed4", [128, T * 4], FP, kind="ExternalInput")
    w2blk = nc.dram_tensor("w2blk", [12, 12], F16, kind="ExternalInput")
    b2c = nc.dram_tensor("b2c", [12, 1], FP, kind="ExternalInput")
    fcwa = nc.dram_tensor("fcwa", [13, 128], F16, kind="ExternalInput")
    h3o = nc.dram_tensor("h3", [128, R_pad], F16, kind="ExternalOutput")
    sto = nc.dram_tensor("bnstat", [128, 2], FP, kind="ExternalOutput")

    GRP = 24
    with tile.TileContext(nc) as tc:
        with tc.tile_pool(name="p", bufs=1) as pool, \
             tc.tile_pool(name="tps", bufs=4, space="PSUM") as tps, \
             tc.tile_pool(name="mms", bufs=2, space="PSUM") as mms, \
             tc.tile_pool(name="fps", bufs=2, space="PSUM") as fps:
            w2b_s = _ld(nc, pool, w2blk, [12, 12], "w2b_s", dt=F16)
            b2_s = _ld(nc, pool, b2c, [12, 1], "b2_s")
            fcw = _ld(nc, pool, fcwa, [13, 128], "fcw", dt=F16)
            ed = pool.tile([128, T, 4], FP, name="ed")
            nc.scalar.dma_start(out=ed[:].rearrange("p t c -> p (t c)"),
                                in_=ed4i.ap())
            ident = pool.tile([128, 128], F16, name="ident")
            make_identity(nc, ident[:])
            rn = pool.tile([128, T, 4], FP, name="rn")
            Mn = pool.tile([128, T, 12], F16, name="Mn")
            MnT = pool.tile([12, R_pad], F16, name="MnT")
            h2T = pool.tile([13, R_pad], F16, name="h2T")
            nc.gpsimd.memset(h2T[:], 1.0)
            h3T = pool.tile([128, R_pad], F16, name="h3T")
            bns = pool.tile([128, 32], FP, name="bns")
            sqa = pool.tile([128, 32], FP, name="sqa")
            nc.vector.memset(bns[:], 0.0)
            nc.vector.memset(sqa[:], 0.0)
            sqs = pool.tile([128, 512], F16, name="sqs")

            M = pool.tile([128, T, 12], FP, name="Macc")
            S_ = pool.tile([128, T, 4], FP, name="Sacc")
            st = {"norm": 0, "tp": 0, "h2": 0, "fc": 0, "dma": 0}

            def tail_cb(hi):
                # 1. reciprocal + normalize for finished tiles
                ta, tz = st["norm"], hi
                if tz > ta:
                    nc.vector.reciprocal(
                        rn[:, ta:tz, :].rearrange("p t h -> p (t h)"),
                        S_[:, ta:tz, :].rearrange("p t h -> p (t h)"))
                    for hf in range(12):
                        h = hf // 3
                        nc.vector.tensor_tensor(
                            out=Mn[:, ta:tz, hf:hf + 1].rearrange(
                                "p t c -> p (t c)"),
                            in0=M[:, ta:tz, hf:hf + 1].rearrange(
                                "p t c -> p (t c)"),
                            in1=rn[:, ta:tz, h:h + 1].rearrange(
                                "p t c -> p (t c)"), op=OP.mult)
                    st["norm"] = tz
                # 2. transposes in 4-tile batches
                while st["tp"] + 4 <= st["norm"] or \
                        (st["norm"] >= T and st["tp"] < T):
                    g0 = st["tp"]
                    g1_ = min(g0 + 4, T)
                    ps = tps.tile([12, 512], F16, tag="tp")
                    for j, t in enumerate(range(g0, g1_)):
                        nc.tensor.transpose(
                            out=ps[:, j * 128:(j + 1) * 128],
                            in_=Mn[:, t, :], identity=ident[:])
                    nc.scalar.activation(
                        out=MnT[0:12, g0 * 128:g1_ * 128],
                        in_=ps[:, 0:(g1_ - g0) * 128], func=AF.Copy)
                    st["tp"] = g1_
                # 3. h2 matmul in 512-col chunks
                tcols = st["tp"] * 128
                while st["h2"] + 512 <= tcols or \
                        (st["tp"] >= T and st["h2"] < R_pad):
                    j0 = st["h2"]
                    j1 = min(j0 + 512, tcols)
                    ps2 = mms.tile([12, j1 - j0], FP, tag="h2m")
                    nc.tensor.matmul(ps2[:], lhsT=w2b_s[:],
                                     rhs=MnT[:, j0:j1], start=True,
                                     stop=True)
                    nc.scalar.activation(out=h2T[0:12, j0:j1], in_=ps2[:],
                                         func=AF.Relu, bias=b2_s[:, 0:1],
                                         scale=1.0)
                    st["h2"] = j1
                if st["h2"] >= R_pad and R_pad > R and not st.get("zr"):
                    nc.gpsimd.memset(h2T[:, R:R_pad], 0.0)
                    st["zr"] = True
                # 4. fc in 512-col chunks; the chunk containing the dummy
                # columns waits until they are zeroed
                lastchunk = (R // 512) * 512
                fcmax = min(st["h2"], lastchunk) if not st.get("zr") \
                    else (st["h2"] if st["h2"] < R_pad else R_pad)
                while st["fc"] + 512 <= fcmax or \
                        (st["h2"] >= R_pad and st["fc"] < R_pad):
                    j0 = st["fc"]
                    j1 = min(j0 + 512, R_pad)
                    psf = fps.tile([128, j1 - j0], FP, tag="fc")
                    nc.tensor.matmul(psf[:], lhsT=fcw[:], rhs=h2T[:, j0:j1],
                                     start=True, stop=True)
                    i = j0 // 512
                    nc.scalar.activation(out=h3T[:, j0:j1], in_=psf[:],
                                         func=AF.Relu,
                                         accum_out=bns[:, i:i + 1])
                    nc.scalar.activation(out=sqs[:, 0:j1 - j0],
                                         in_=h3T[:, j0:j1], func=AF.Square,
                                         accum_out=sqa[:, i:i + 1])
                    st["fc"] = j1
                # 5. h3 write-out in 2048-col pieces
                while st["dma"] + 2048 <= st["fc"] or \
                        (st["fc"] >= R_pad and st["dma"] < R_pad):
                    j0 = st["dma"]
                    j1 = min(j0 + 2048, R_pad)
                    nc.sync.dma_start(out=h3o.ap()[:, j0:j1],
                                      in_=h3T[:, j0:j1])
                    st["dma"] = j1

            _edge_phase(nc, tc, pool, meta, g2, ed, 3, 4,
                        pool_prod=POOL_PROD_L2, chunk_done_cb=tail_cb,
                        M=M, S_=S_, tb_pool=TB_POOL)

            st2 = pool.tile([128, 2], FP, name="st2")
            nc.vector.tensor_reduce(out=st2[:, 0:1], in_=bns[:],
                                    axis=mybir.AxisListType.X, op=OP.add)
            nc.vector.tensor_reduce(out=st2[:, 1:2], in_=sqa[:],
                                    axis=mybir.AxisListType.X, op=OP.add)
            nc.sync.dma_start(out=sto.ap(), in_=st2[:])
    nc.compile()
    return nc


def _build_l3(meta, n):
    nc = bacc.Bacc("TRN2", target_bir_lowering=False, debug=False,
                   num_devices=N_CORES)
    R_pad = meta["R_pad"]
    h3i = nc.dram_tensor("h3", [128, R_pad], F16, kind="ExternalInput")
    sti = nc.dram_tensor("bnstats", [128, 16], FP, kind="ExternalInput")
    bng = nc.dram_tensor("bng", [128, 1], FP, kind="ExternalInput")
    bnb = nc.dram_tensor("bnb", [128, 1], FP, kind="ExternalInput")
    l2wa = nc.dram_tensor("l2wa", [128, 64], F16, kind="ExternalInput")
    l2bb = nc.dram_tensor("l2bb", [64, 1], FP, kind="ExternalInput")
    owa = nc.dram_tensor("owa", [65, 6], F16, kind="ExternalInput")
    outo = nc.dram_tensor("out", [6, R_pad], FP, kind="ExternalOutput")

    with tile.TileContext(nc) as tc:
        with tc.tile_pool(name="p", bufs=1) as pool, \
             tc.tile_pool(name="ps", bufs=4, space="PSUM") as pp:
            sts = _ld(nc, pool, sti, [128, 16], "sts")
            h3s = pool.tile([128, R_pad], F16, name="h3s")
            q = R_pad // 8
            for j in range(0, R_pad, q):
                nc.scalar.dma_start(out=h3s[:, j:j + q],
                                    in_=h3i.ap()[:, j:j + q])
            bng_s = _ld(nc, pool, bng, [128, 1], "bng_s")
            bnb_s = _ld(nc, pool, bnb, [128, 1], "bnb_s")
            l2w = _ld(nc, pool, l2wa, [128, 64], "l2w", dt=F16)
            l2b = _ld(nc, pool, l2bb, [64, 1], "l2b")
            ow = _ld(nc, pool, owa, [65, 6], "ow", dt=F16)

            red = pool.tile([128, 2], FP, name="red")
            nc.vector.tensor_reduce(
                out=red[:], in_=sts[:].rearrange("p (s c) -> p s c", s=2),
                axis=mybir.AxisListType.X, op=OP.add)
            mu = pool.tile([128, 1], FP, name="mu")
            nc.vector.tensor_scalar(out=mu[:], in0=red[:, 0:1],
                                    scalar1=1.0 / n, scalar2=None,
                                    op0=OP.mult)
            m2 = pool.tile([128, 1], FP, name="m2")
            nc.vector.tensor_scalar(out=m2[:], in0=red[:, 1:2],
                                    scalar1=1.0 / n, scalar2=None,
                                    op0=OP.mult)
            var = pool.tile([128, 1], FP, name="var")
            nc.vector.tensor_tensor(out=var[:], in0=mu[:], in1=mu[:],
                                    op=OP.mult)
            nc.vector.tensor_tensor(out=var[:], in0=m2[:], in1=var[:],
                                    op=OP.subtract)
            epsb = pool.tile([128, 1], FP, name="epsb")
            nc.vector.memset(epsb[:], BN_EPS)
            sd = pool.tile([128, 1], FP, name="sd")
            nc.scalar.activation(out=sd[:], in_=var[:], func=AF.Sqrt,
                                 bias=epsb[:], scale=1.0)
            rsig = pool.tile([128, 1], FP, name="rsig")
            nc.vector.reciprocal(rsig[:], sd[:])
            scale = pool.tile([128, 1], FP, name="scale")
            nc.vector.tensor_tensor(out=scale[:], in0=bng_s[:], in1=rsig[:],
                                    op=OP.mult)
            shift = pool.tile([128, 1], FP, name="shift")
            nc.vector.tensor_tensor(out=shift[:], in0=mu[:], in1=scale[:],
                                    op=OP.mult)
            nc.vector.tensor_tensor(out=shift[:], in0=bnb_s[:], in1=shift[:],
                                    op=OP.subtract)

            hbn = pool.tile([128, R_pad], F16, name="hbn")
            h4a = pool.tile([65, R_pad], F16, name="h4a")
            nc.gpsimd.memset(h4a[:], 1.0)
            outT = pool.tile([6, R_pad], FP, name="outT")
            chunks = [(j, min(j + 512, R_pad)) for j in range(0, R_pad, 512)]
            for (j0, j1) in chunks:
                nc.vector.tensor_scalar(out=hbn[:, j0:j1], in0=h3s[:, j0:j1],
                                        scalar1=scale[:], scalar2=shift[:],
                                        op0=OP.mult, op1=OP.add)
                ps = pp.tile([64, j1 - j0], FP, tag="l2")
                nc.tensor.matmul(ps[:], lhsT=l2w[:], rhs=hbn[:, j0:j1],
                                 start=True, stop=True)
                nc.vector.tensor_scalar(out=h4a[0:64, j0:j1], in0=ps[:],
                                        scalar1=l2b[:, 0:1], scalar2=None,
                                        op0=OP.add)
                ps2 = pp.tile([6, j1 - j0], FP, tag="out")
                nc.tensor.matmul(ps2[:], lhsT=ow[:], rhs=h4a[:, j0:j1],
                                 start=True, stop=True)
                nc.scalar.activation(out=outT[:, j0:j1], in_=ps2[:],
                                     func=AF.Sigmoid)
            nc.sync.dma_start(out=outo.ap(), in_=outT[:])
    nc.compile()
    return nc


# --------------------------------------------------------------------------
# Orchestration
# --------------------------------------------------------------------------

def _bcast(a, cols):
    return np.ascontiguousarray(np.broadcast_to(
        np.asarray(a, np.float32).reshape(1, -1), (128, cols)))


def _run(nc, in_maps):
    import time as _t
    t0 = _t.perf_counter()
    res = run_bass_kernel_spmd(nc, in_maps, list(range(N_CORES)))
    LAUNCH_WALL.append(_t.perf_counter() - t0)
    LAST_RESULTS.append(res)
    return res.results


def _rows_to_pernode(meta, arrs):
    R = meta["R"]
    F = arrs[0].shape[1]
    out = np.empty((R * N_CORES, F), arrs[0].dtype)
    for c in range(N_CORES):
        out[meta["rows_node"][c]] = arrs[c][:R]
    return out


def EXTRA_TSIM_BUILDERS(meta, n):
    return {
        "l0": lambda: _build_l0(meta),
        "l1": lambda: _build_l1(meta),
        "l2": lambda: _build_l2(meta, n),
        "l3": lambda: _build_l3(meta, n),
    }


def kernel(x, edge_index, W1, a_src1, a_dst1, b1, W2, a_src2, a_dst2, b2,
           fc_W, fc_b, bn_g, bn_b, l2_W, l2_b, out_W, out_b):
    global LAST_RESULTS
    LAST_RESULTS = []
    x = np.asarray(x, np.float32)
    n = x.shape[0]
    ekey = (n, np.asarray(edge_index).shape[1])
    meta = _PROG_CACHE.get(("meta", ekey))
    fp = np.asarray(edge_index)[:, :: max(1, ekey[1] // 64)]
    if meta is None or not np.array_equal(meta["_fp"], fp):
        meta = _preprocess(np.asarray(edge_index), n)
        meta["_fp"] = fp.copy()
        _PROG_CACHE.clear()
        _PROG_CACHE[("meta", ekey)] = meta
    R, R_pad = meta["R"], meta["R_pad"]
    if ("l0", ekey) not in _PROG_CACHE:
        _PROG_CACHE[("l0", ekey)] = _build_l0(meta)
        _PROG_CACHE[("l1", ekey)] = _build_l1(meta)
        _PROG_CACHE[("l2", ekey)] = _build_l2(meta, n)
        _PROG_CACHE[("l3", ekey)] = _build_l3(meta, n)

    # ---- launch 0: per-node u / es1 / ed1
    T = meta["T"]

    def to_dev(a):      # [R_pad, C] rank-major -> [128, T*C]
        C = a.shape[1]
        return np.ascontiguousarray(
            a.reshape(T, 128, C).transpose(1, 0, 2).reshape(128, T * C))

    def from_dev(a, C):  # [128, T*C] -> [R_pad, C] rank-major
        return a.reshape(128, T, C).transpose(1, 0, 2).reshape(R_pad, C)

    tail15 = np.concatenate([
        np.asarray(W1, np.float32).reshape(-1),
        np.asarray(a_src1, np.float32).reshape(-1),
        np.asarray(a_dst1, np.float32).reshape(-1)]).reshape(1, 15)
    in_maps = []
    for c in range(N_CORES):
        xr = np.zeros((R_pad, 3), np.float32)
        xr[:R] = x[meta["rows_node"][c]]
        # c-major planes [x0(T) x1(T) x2(T)] + [W1|a_src1|a_dst1]
        xrd = xr.reshape(T, 128, 3).transpose(1, 2, 0).reshape(128, 3 * T)
        in_maps.append(dict(
            xr=np.ascontiguousarray(np.concatenate(
                [xrd, np.broadcast_to(tail15, (128, 15))], axis=1))))
    r0 = _run(_PROG_CACHE[("l0", ekey)], in_maps)

    def u_unpack(a):   # [128, 4T] c-major -> [R_pad, 4] rank-major
        return a.reshape(128, 4, T).transpose(2, 0, 1).reshape(R_pad, 4)

    u4 = [u_unpack(r0[c]["u"]) for c in range(N_CORES)]
    u_pn = _rows_to_pernode(meta, [a[:, 0:3] for a in u4])
    es1_pn = _rows_to_pernode(meta, [a[:, 3:4] for a in u4])

    # ---- launch 1
    g1 = _make_stream(meta, u_pn, es1_pn)
    in_maps = []
    for c in range(N_CORES):
        in_maps.append(dict(
            g1=g1[c], ed1=np.ascontiguousarray(r0[c]["ed1"]),
            b1b=_bcast(b1, 3), w2b=_bcast(W2, 36), as2=_bcast(a_src2, 12),
            ad2=_bcast(a_dst2, 12)))
    r1 = _run(_PROG_CACHE[("l1", ekey)], in_maps)

    x2_pn = _rows_to_pernode(meta, [from_dev(r1[c]["x2"], 3)
                                    for c in range(N_CORES)])
    es4_pn = _rows_to_pernode(meta, [from_dev(r1[c]["es4"], 4)
                                    for c in range(N_CORES)])

    # ---- launch 2 (GAT2 + fc + BN partial stats)
    g2 = _make_stream(meta, x2_pn, es4_pn)
    fcwa = np.vstack([np.asarray(fc_W, np.float32),
                      np.asarray(fc_b, np.float32)[None, :]]).astype(
        np.float16)
    # block-diagonal per-head W2: w2blk[(h,f), hc] = W2[f, hc] iff hc//3 == h
    W2f = np.asarray(W2, np.float32)
    w2blk = np.zeros((12, 12), np.float32)
    for h in range(4):
        w2blk[h * 3:(h + 1) * 3, h * 3:(h + 1) * 3] = W2f[:, h * 3:(h + 1) * 3]
    w2blk = w2blk.astype(np.float16)
    in_maps = []
    for c in range(N_CORES):
        in_maps.append(dict(
            g2=g2[c], ed4=np.ascontiguousarray(r1[c]["ed4"]),
            w2blk=w2blk, b2c=np.asarray(b2, np.float32).reshape(12, 1),
            fcwa=fcwa))
    r2 = _run(_PROG_CACHE[("l2", ekey)], in_maps)

    stats = np.zeros((128, 16), np.float32)
    for c in range(N_CORES):
        stats[:, c] = r2[c]["bnstat"][:, 0]
        stats[:, 8 + c] = r2[c]["bnstat"][:, 1]

    # ---- launch 3 (BN finalize/apply + MLP output)
    l2wa = np.asarray(l2_W, np.float32).astype(np.float16)
    owa = np.vstack([np.asarray(out_W, np.float32),
                     np.asarray(out_b, np.float32)[None, :]]).astype(
        np.float16)
    in_maps = []
    for c in range(N_CORES):
        in_maps.append(dict(
            h3=r2[c]["h3"], bnstats=stats,
            bng=np.asarray(bn_g, np.float32).reshape(128, 1),
            bnb=np.asarray(bn_b, np.float32).reshape(128, 1),
            l2wa=l2wa,
            l2bb=np.asarray(l2_b, np.float32).reshape(64, 1),
            owa=owa))
    r3 = _run(_PROG_CACHE[("l3", ekey)], in_maps)

    out = np.zeros((n, 6), np.float32)
    for c in range(N_CORES):
        out[meta["rows_node"][c]] = r3[c]["out"][:, :R].T
    return out


# revision 4
# speedup vs baseline: 1.0092x; 1.0036x over previous
"""GATConv x2 + MLP head GNN over 8 Trainium2 cores — cost-model-tuned v2.

Structure (4 SPMD launches; host does only index manipulation / gathers):
  l0: per-node u = x@W1 (fp16), es1/ed1 attention terms.
  l1: GAT layer 1 over fp16 slot stream [u0 u1 u2 es1]; per-tile
      tensor_scalar tb, big stt lrelu, big ACT exp, fused stt product+accum
      for M (split DVE/gpsimd), ts+accum for s.  Tail: x2, es4, ed4.
  l2: GAT layer 2 over fp16 stream [x2(3) es4(4)] (same shape as l1 but
      4 heads); tail: h2, PE transpose -> h2T, fc matmul (bias via aug
      ones row), BN partial stats. Outputs h3T bf16 + stats.
  l3: BN finalize/apply, two bf16 matmuls + sigmoid, outT [6, R_pad].
Slot streams are packed per chunk of tiles: [128, npl, CH] fp16 with
per-tile column ranges; pad slots carry es=-3e38 (exp->0), dummy rows get
one live zero slot so s>0.
"""

import numpy as np
import ml_dtypes

import concourse.bass as bass
import concourse.bacc as bacc
import concourse.tile as tile
from concourse import mybir
from concourse.bass_utils import run_bass_kernel_spmd
from concourse.masks import make_identity

FP = mybir.dt.float32
F16 = mybir.dt.float16
BF = mybir.dt.bfloat16
AF = mybir.ActivationFunctionType
OP = mybir.AluOpType

N_CORES = 8
SLOPE = 0.2
BN_EPS = 1e-5
NEG_BIG = -60000.0  # finite in fp16; exp() still underflows to exactly 0
import os as _os
CH_MAX = int(_os.environ.get('CHM', '768'))
POOL_PROD_L1 = int(_os.environ.get('PPL1', '2'))
POOL_PROD_L2 = int(_os.environ.get('PPL2', '6'))
TB_POOL = int(_os.environ.get('TBP', '2'))
TB_POOL_L1 = int(_os.environ.get('TBPL1', '0'))

_PROG_CACHE = {}
LAST_RESULTS = []
LAUNCH_WALL = []


# --------------------------------------------------------------------------
# Host-side preprocessing (index manipulation only)
# --------------------------------------------------------------------------

def _preprocess(edge_index, n):
    src = np.asarray(edge_index[0], dtype=np.int64)
    dst = np.asarray(edge_index[1], dtype=np.int64)
    loops = np.arange(n, dtype=np.int64)
    src = np.concatenate([src, loops])
    dst = np.concatenate([dst, loops])

    assert n % N_CORES == 0
    R = n // N_CORES
    T = -(-R // 128)
    R_pad = T * 128

    owner = dst // R
    per_core = []
    degs = []
    for c in range(N_CORES):
        m = owner == c
        s_c = src[m]
        d_loc = dst[m] - c * R
        deg = np.bincount(d_loc, minlength=R)
        row_of = np.argsort(-deg, kind="stable")
        per_core.append((s_c, d_loc, deg[row_of], row_of))
        degs.append(deg[row_of])

    tile_k = np.zeros(T, dtype=np.int64)
    for t in range(T):
        lo, hi = t * 128, min(t * 128 + 128, R)
        kmax = 1
        if hi > lo:
            for c in range(N_CORES):
                kmax = max(kmax, int(degs[c][lo:hi].max()))
        tile_k[t] = -(-max(kmax, 1) // 4) * 4
    tile_off = np.concatenate([[0], np.cumsum(tile_k * 128)])
    S = int(tile_off[-1])

    # chunks of tiles with total column budget (ramped up at the start
    # so the first compute can begin after a small DMA)
    chunks = []
    t0 = 0
    budgets = [CH_MAX // 4, CH_MAX // 2]
    while t0 < T:
        bud = budgets[len(chunks)] if len(chunks) < len(budgets) else CH_MAX
        t1, ch = t0, 0
        while t1 < T and (t1 == t0 or ch + tile_k[t1] <= bud):
            ch += tile_k[t1]
            t1 += 1
        chunks.append((t0, t1, int(ch)))
        t0 = t1

    # slot -> source node (or -1 pad / -2 dummy-live), canonical numbering
    # pos = tile_off[t] + p*K_t + j
    slots_all = np.full((N_CORES, S), -1, dtype=np.int64)
    rows_node = np.empty((N_CORES, R), dtype=np.int64)
    for c in range(N_CORES):
        s_c, d_loc, deg_sorted, row_of = per_core[c]
        rank_of = np.empty(R, dtype=np.int64)
        rank_of[row_of] = np.arange(R)
        rows_node[c] = row_of + c * R

        erow = rank_of[d_loc]
        eorder = np.argsort(erow, kind="stable")
        erow_s = erow[eorder]
        esrc_s = s_c[eorder]
        row_start = np.concatenate([[0], np.cumsum(deg_sorted)])
        slot_no = np.arange(len(erow_s)) - row_start[erow_s]
        et = erow_s // 128
        ep = erow_s % 128
        pos = tile_off[et] + ep * tile_k[et] + slot_no
        slots_all[c][pos] = esrc_s
    # dummy rows (rank >= R): slot 0 is a live zero-contribution slot
    for r in range(R, R_pad):
        t = r // 128
        p = r % 128
        slots_all[:, tile_off[t] + p * tile_k[t]] = -2

    return dict(
        R=R, R_pad=R_pad, T=T, S=S,
        tile_k=[int(k) for k in tile_k],
        tile_off=[int(o) for o in tile_off],
        chunks=chunks, slots=slots_all, rows_node=rows_node,
    )


def _perm(meta, npl):
    """stream position -> flat index into [S, npl] (slot-major gather)."""
    key = ("perm", npl)
    p = meta.get(key)
    if p is not None:
        return p
    tile_k = meta["tile_k"]
    tile_off = meta["tile_off"]
    parts = []
    p_i = np.arange(128)[:, None, None]
    c_i = np.arange(npl)[None, :, None]
    for (t0, t1, CH) in meta["chunks"]:
        cols = []
        for t in range(t0, t1):
            K = tile_k[t]
            k_i = np.arange(K)[None, None, :]
            cols.append((tile_off[t] + p_i * K + k_i) * npl + c_i)
        parts.append(np.concatenate(cols, axis=2).reshape(-1))
    p = np.concatenate(parts)
    meta[key] = p
    return p


def _make_stream(meta, pernode, es_pernode):
    """Per-core fp16 slot stream, chunk-major [128, npl, CH] layout."""
    F = pernode.shape[1]
    H = es_pernode.shape[1]
    n = pernode.shape[0]
    npl = F + H
    perm = _perm(meta, npl)
    ptab = np.vstack([pernode.astype(np.float32),
                      np.zeros((2, F), np.float32)])
    etab = np.vstack([es_pernode.astype(np.float32),
                      np.full((1, H), NEG_BIG, np.float32),
                      np.zeros((1, H), np.float32)])
    out = np.empty((N_CORES, meta["S"] * npl), dtype=np.float16)
    for c in range(N_CORES):
        sl = meta["slots"][c].copy()
        sl[sl == -1] = n
        sl[sl == -2] = n + 1
        both = np.concatenate([ptab[sl], etab[sl]], axis=1)
        out[c] = both.reshape(-1)[perm].astype(np.float16)
    return out


# --------------------------------------------------------------------------
# Device program helpers
# --------------------------------------------------------------------------

def _ld(nc, pool, dram, shape, name, dt=FP):
    t = pool.tile(shape, dt, name=name)
    nc.sync.dma_start(out=t[:], in_=dram.ap())
    return t


def _vpair(nc, pool, wb, ab, nch, nheads, name):
    """v[f,h] = sum_c W[f, h*nch+c] * a[h*nch+c] -> [128, nch, nheads]."""
    vt = pool.tile([128, nch, nheads], FP, name=name)
    for h in range(nheads):
        for cc in range(nch):
            o = vt[:, :, h:h + 1].rearrange("p f h -> p (f h)")
            wcols = wb[:].rearrange("p (f hc) -> p f hc", f=nch)[
                :, :, h * nch + cc]
            sc = ab[:, h * nch + cc:h * nch + cc + 1]
            if cc == 0:
                nc.vector.tensor_scalar(out=o, in0=wcols, scalar1=sc,
                                        scalar2=None, op0=OP.mult)
            else:
                nc.vector.scalar_tensor_tensor(out=o, in0=wcols, scalar=sc,
                                               in1=o, op0=OP.mult, op1=OP.add)
    return vt


def _rows_affine(nc, pool, T, xr, vt, nch, nheads, name, dt=FP):
    """out[p, t, h] = sum_f xr[p, t, f] * vt[f, h]."""
    out = pool.tile([128, T, nheads], dt, name=name)
    for h in range(nheads):
        o = out[:, :, h:h + 1].rearrange("p t h -> p (t h)")
        for f in range(nch):
            srcv = xr[:, :, f:f + 1].rearrange("p t f -> p (t f)")
            sc = vt[:, f, h:h + 1].rearrange("p h -> p h")
            if f == 0:
                nc.vector.tensor_scalar(out=o, in0=srcv, scalar1=sc,
                                        scalar2=None, op0=OP.mult)
            else:
                nc.vector.scalar_tensor_tensor(out=o, in0=srcv, scalar=sc,
                                               in1=o, op0=OP.mult, op1=OP.add)
    return out


def _edge_phase(nc, tc, pool, meta, g_dram, ed_rows, nch, nheads, pool_prod,
                chunk_done_cb=None, M=None, S_=None, tb_pool=0):
    """Edge aggregation over the packed fp16 chunk stream.

    g_dram: [S*npl] fp16, npl = nch + nheads, planes [x.. | es..].
    ed_rows: SBUF [128, T, nheads] fp32 per-row dst logits.
    pool_prod: planes hf >= nheads*nch - pool_prod compute their e*x product
    on the gpsimd engine (plain tensor_tensor), with a cheap fp16
    tensor_scalar+accum on DVE; the rest use a fused DVE stt+accum.
    Returns (M [128, T, nheads*nch] fp32, S_ [128, T, nheads] fp32).
    """
    T = meta["T"]
    tile_k = meta["tile_k"]
    npl = nch + nheads
    if M is None:
        M = pool.tile([128, T, nheads * nch], FP, name="Macc")
    if S_ is None:
        S_ = pool.tile([128, T, nheads], FP, name="Sacc")
    scd = pool.tile([128, CH_MAX], F16, name="scd")

    npm = nheads * nch
    prb = None
    if pool_prod:
        prb = [pool.tile([128, pool_prod, CH_MAX], F16, name=f"prb{i}")
               for i in range(2)]

    def emit_accums(ci, t0, t1):
        """DVE ts+accum over the gpsimd products of chunk ci."""
        co = 0
        for t in range(t0, t1):
            K = tile_k[t]
            for j in range(pool_prod):
                hf = npm - pool_prod + j
                nc.vector.tensor_scalar(
                    out=scd[:, co:co + K],
                    in0=prb[ci % 2][:, j, co:co + K],
                    scalar1=1.0, scalar2=0.0, op0=OP.mult,
                    op1=OP.add, accum_out=M[:, t, hf:hf + 1])
            co += K

    with tc.tile_pool(name="edge", bufs=3) as ep, \
         tc.tile_pool(name="edge2", bufs=2) as e2:
        deferred = None
        for ci, (t0, t1, CH) in enumerate(meta["chunks"]):
            off0 = meta["tile_off"][t0] * npl
            off1 = meta["tile_off"][t1] * npl
            G = ep.tile([128, npl, CH], F16, tag="G")
            nc.sync.dma_start(
                out=G[:].rearrange("p c x -> p (c x)"),
                in_=g_dram.ap()[off0:off1].rearrange("(p x) -> p x", p=128))
            tb = e2.tile([128, nheads, CH], F16, tag="tb")
            lr = e2.tile([128, nheads, CH], F16, tag="lr")
            e = e2.tile([128, nheads, CH], BF, tag="e")

            tmid = (t0 + t1 + 1) // 2
            com = sum(tile_k[t] for t in range(t0, tmid))
            halves = [(t0, tmid, 0, com), (tmid, t1, com, CH)]

            def emit_tb(ta, tz, ca):
                co = ca
                for t in range(ta, tz):
                    K = tile_k[t]
                    for h in range(nheads):
                        eng = nc.gpsimd if h < tb_pool else nc.vector
                        eng.tensor_scalar(
                            out=tb[:, h, co:co + K],
                            in0=G[:, nch + h, co:co + K],
                            scalar1=ed_rows[:, t, h:h + 1],
                            scalar2=None, op0=OP.add)
                    co += K

            def emit_act(ca, cz):
                nc.scalar.activation(
                    out=lr[:, :, ca:cz], in_=tb[:, :, ca:cz],
                    func=AF.Prelu, alpha=SLOPE)
                nc.scalar.activation(
                    out=e[:, :, ca:cz], in_=lr[:, :, ca:cz],
                    func=AF.Exp)

            def emit_prod(ta, tz, ca):
                co = ca
                for t in range(ta, tz):
                    K = tile_k[t]
                    for j in range(pool_prod):
                        hf = npm - pool_prod + j
                        h, f = hf // nch, hf % nch
                        nc.gpsimd.tensor_tensor(
                            out=prb[ci % 2][:, j, co:co + K],
                            in0=e[:, h, co:co + K],
                            in1=G[:, f, co:co + K], op=OP.mult)
                    co += K

            def emit_stts(ta, tz, ca):
                co = ca
                for t in range(ta, tz):
                    K = tile_k[t]
                    for h in range(nheads):
                        for f in range(nch):
                            hf = h * nch + f
                            if hf < npm - pool_prod:
                                nc.vector.scalar_tensor_tensor(
                                    out=scd[:, co:co + K],
                                    in0=e[:, h, co:co + K],
                                    scalar=1.0, in1=G[:, f, co:co + K],
                                    op0=OP.bypass, op1=OP.mult,
                                    accum_out=M[:, t, hf:hf + 1])
                        nc.vector.tensor_scalar(
                            out=scd[:, co:co + K], in0=e[:, h, co:co + K],
                            scalar1=1.0, scalar2=0.0, op0=OP.mult,
                            op1=OP.add, accum_out=S_[:, t, h:h + 1])
                    co += K

            (ta0, tz0, ca0, cz0), (ta1, tz1, ca1, cz1) = halves
            emit_tb(ta0, tz0, ca0)
            emit_act(ca0, cz0)         # ACT half A starts
            emit_tb(ta1, tz1, ca1)     # DVE does half-B tb meanwhile
            if pool_prod and deferred is not None:
                emit_accums(*deferred)  # ready DVE work during ACT latency
            if cz1 > ca1:
                emit_act(ca1, cz1)
            emit_prod(ta0, tz0, ca0)
            emit_stts(ta0, tz0, ca0)
            emit_prod(ta1, tz1, ca1)
            emit_stts(ta1, tz1, ca1)
            if chunk_done_cb is not None:
                done_hi = meta["chunks"][ci - 1][1] if pool_prod else t1
                if deferred is None and pool_prod:
                    done_hi = 0
                chunk_done_cb(done_hi)
            deferred = (ci, t0, t1)
        if pool_prod and deferred is not None:
            emit_accums(*deferred)
        if chunk_done_cb is not None:
            chunk_done_cb(meta["T"])
    return M, S_


def _gat_head_out(nc, pool, T, M, S_, wb, bb, nch, nheads, name, dt=F16):
    """out[:, t, hc] = relu((sum_f M[h,f] W[f,hc]) / s_h + b[hc]).

    wb None => stream carried pre-projected features (M_hf is already the
    per-output-channel aggregate): out_hc = relu(M_hc / s_h + b[hc])."""
    rn = pool.tile([128, T, nheads], FP, name=name + "rn")
    nc.vector.reciprocal(rn[:].rearrange("p t h -> p (t h)"),
                         S_[:].rearrange("p t h -> p (t h)"))
    out = pool.tile([128, T, nheads * nch], dt, name=name)
    t1 = pool.tile([128, T], FP, name=name + "t1")
    t2 = pool.tile([128, T], FP, name=name + "t2")
    for h in range(nheads):
        for c in range(nch):
            hc = h * nch + c
            if wb is None:
                t1v = M[:, :, hc:hc + 1].rearrange("p t c -> p (t c)")
            else:
                for f in range(nch):
                    srcv = M[:, :, h * nch + f:h * nch + f + 1].rearrange(
                        "p t c -> p (t c)")
                    wsc = wb[:].rearrange("p (f hc) -> p f hc", f=nch)[
                        :, f, hc:hc + 1]
                    if f == 0:
                        nc.vector.tensor_scalar(
                            out=t1[:], in0=srcv, scalar1=wsc,
                            scalar2=None, op0=OP.mult)
                    else:
                        nc.vector.scalar_tensor_tensor(
                            out=t1[:], in0=srcv, scalar=wsc, in1=t1[:],
                            op0=OP.mult, op1=OP.add)
                t1v = t1[:]
            nc.vector.tensor_tensor(
                out=t2[:], in0=t1v,
                in1=rn[:, :, h:h + 1].rearrange("p t h -> p (t h)"),
                op=OP.mult)
            nc.vector.tensor_scalar(
                out=out[:, :, hc:hc + 1].rearrange("p t c -> p (t c)"),
                in0=t2[:], scalar1=bb[:, hc:hc + 1], scalar2=0.0,
                op0=OP.add, op1=OP.max)
    return out


# --------------------------------------------------------------------------
# Launch builders
# --------------------------------------------------------------------------

def _build_l0(meta):
    nc = bacc.Bacc("TRN2", target_bir_lowering=False, debug=False,
                   num_devices=N_CORES)
    T = meta["T"]
    B = 3 * T
    xr_i = nc.dram_tensor("xr", [128, B + 15], FP, kind="ExternalInput")
    uo = nc.dram_tensor("u", [128, 4 * T], F16, kind="ExternalOutput")
    ed1o = nc.dram_tensor("ed1", [128, T], FP, kind="ExternalOutput")

    with tile.TileContext(nc) as tc:
        with tc.tile_pool(name="p", bufs=1) as pool:
            allin = pool.tile([128, B + 15], FP, name="allin")
            nc.sync.dma_start(out=allin[:], in_=xr_i.ap())
            u = pool.tile([128, 4, T], F16, name="u")
            ed1 = pool.tile([128, T], FP, name="ed1")
            # u[:, f, :] = sum_c x_c * W1[c, f]
            for f in range(3):
                o = u[:, f, :]
                for c in range(3):
                    srcv = allin[:, c * T:(c + 1) * T]
                    w0 = B + c * 3 + f
                    if c == 0:
                        nc.vector.tensor_scalar(
                            out=o, in0=srcv, scalar1=allin[:, w0:w0 + 1],
                            scalar2=None, op0=OP.mult)
                    else:
                        nc.vector.scalar_tensor_tensor(
                            out=o, in0=srcv, scalar=allin[:, w0:w0 + 1],
                            in1=o, op0=OP.mult, op1=OP.add)
            # es1 (u plane 3) and ed1: dot of u planes with a_src1/a_dst1
            for f in range(3):
                a0 = B + 9 + f
                d0 = B + 12 + f
                if f == 0:
                    nc.vector.tensor_scalar(
                        out=u[:, 3, :], in0=u[:, 0, :],
                        scalar1=allin[:, a0:a0 + 1], scalar2=None,
                        op0=OP.mult)
                    nc.vector.tensor_scalar(
                        out=ed1[:], in0=u[:, 0, :],
                        scalar1=allin[:, d0:d0 + 1], scalar2=None,
                        op0=OP.mult)
                else:
                    nc.vector.scalar_tensor_tensor(
                        out=u[:, 3, :], in0=u[:, f, :],
                        scalar=allin[:, a0:a0 + 1], in1=u[:, 3, :],
                        op0=OP.mult, op1=OP.add)
                    nc.vector.scalar_tensor_tensor(
                        out=ed1[:], in0=u[:, f, :],
                        scalar=allin[:, d0:d0 + 1], in1=ed1[:],
                        op0=OP.mult, op1=OP.add)
            nc.sync.dma_start(out=uo.ap(),
                              in_=u[:].rearrange("p c t -> p (c t)"))
            nc.sync.dma_start(out=ed1o.ap(), in_=ed1[:])
    nc.compile()
    return nc


def _build_l1(meta):
    nc = bacc.Bacc("TRN2", target_bir_lowering=False, debug=False,
                   num_devices=N_CORES)
    T, S, R_pad = meta["T"], meta["S"], meta["R_pad"]
    g1 = nc.dram_tensor("g1", [S * 4], F16, kind="ExternalInput")
    ed1i = nc.dram_tensor("ed1", [128, T], FP, kind="ExternalInput")
    b1b = nc.dram_tensor("b1b", [128, 3], FP, kind="ExternalInput")
    w2b = nc.dram_tensor("w2b", [128, 36], FP, kind="ExternalInput")
    as2 = nc.dram_tensor("as2", [128, 12], FP, kind="ExternalInput")
    ad2 = nc.dram_tensor("ad2", [128, 12], FP, kind="ExternalInput")
    x2o = nc.dram_tensor("x2", [128, T * 3], F16, kind="ExternalOutput")
    es4o = nc.dram_tensor("es4", [128, T * 4], F16, kind="ExternalOutput")
    ed4o = nc.dram_tensor("ed4", [128, T * 4], FP, kind="ExternalOutput")

    with tile.TileContext(nc) as tc:
        with tc.tile_pool(name="p", bufs=1) as pool:
            b_b = _ld(nc, pool, b1b, [128, 3], "b_b")
            w2 = _ld(nc, pool, w2b, [128, 36], "w2")
            a_s2 = _ld(nc, pool, as2, [128, 12], "a_s2")
            a_d2 = _ld(nc, pool, ad2, [128, 12], "a_d2")
            ed = pool.tile([128, T, 1], FP, name="ed")
            nc.sync.dma_start(out=ed[:].rearrange("p t c -> p (t c)"),
                              in_=ed1i.ap())
            M, S_ = _edge_phase(nc, tc, pool, meta, g1, ed, 3, 1,
                                pool_prod=POOL_PROD_L1, tb_pool=TB_POOL_L1)
            x2 = _gat_head_out(nc, pool, T, M, S_, None, b_b, 3, 1, "x2",
                               dt=F16)
            vs2 = _vpair(nc, pool, w2, a_s2, 3, 4, "vs2")
            vd2 = _vpair(nc, pool, w2, a_d2, 3, 4, "vd2")
            es4 = _rows_affine(nc, pool, T, x2, vs2, 3, 4, "es4r", dt=F16)
            ed4 = _rows_affine(nc, pool, T, x2, vd2, 3, 4, "ed4r")
            nc.sync.dma_start(out=x2o.ap(),
                              in_=x2[:].rearrange("p t c -> p (t c)"))
            nc.sync.dma_start(out=es4o.ap(),
                              in_=es4[:].rearrange("p t c -> p (t c)"))
            nc.sync.dma_start(out=ed4o.ap(),
                              in_=ed4[:].rearrange("p t c -> p (t c)"))
    nc.compile()
    return nc


def _build_l2(meta, n):
    nc = bacc.Bacc("TRN2", target_bir_lowering=False, debug=False,
                   num_devices=N_CORES)
    T, S, R_pad, R = meta["T"], meta["S"], meta["R_pad"], meta["R"]
    g2 = nc.dram_tensor("g2", [S * 7], F16, kind="ExternalInput")
    ed4i = nc.dram_tensor("ed4", [128, T * 4], FP, kind="ExternalInput")
    w2blk = nc.dram_tensor("w2blk", [12, 12], F16, kind="ExternalInput")
    b2c = nc.dram_tensor("b2c", [12, 1], FP, kind="ExternalInput")
    fcwa = nc.dram_tensor("fcwa", [13, 128], F16, kind="ExternalInput")
    h3o = nc.dram_tensor("h3", [128, R_pad], F16, kind="ExternalOutput")
    sto = nc.dram_tensor("bnstat", [128, 2], FP, kind="ExternalOutput")

    GRP = 24
    with tile.TileContext(nc) as tc:
        with tc.tile_pool(name="p", bufs=1) as pool, \
             tc.tile_pool(name="tps", bufs=4, space="PSUM") as tps, \
             tc.tile_pool(name="mms", bufs=2, space="PSUM") as mms, \
             tc.tile_pool(name="fps", bufs=2, space="PSUM") as fps:
            w2b_s = _ld(nc, pool, w2blk, [12, 12], "w2b_s", dt=F16)
            b2_s = _ld(nc, pool, b2c, [12, 1], "b2_s")
            fcw = _ld(nc, pool, fcwa, [13, 128], "fcw", dt=F16)
            ed = pool.tile([128, T, 4], FP, name="ed")
            nc.scalar.dma_start(out=ed[:].rearrange("p t c -> p (t c)"),
                                in_=ed4i.ap())
            ident = pool.tile([128, 128], F16, name="ident")
            make_identity(nc, ident[:])
            rn = pool.tile([128, T, 4], FP, name="rn")
            Mn = pool.tile([128, T, 12], F16, name="Mn")
            MnT = pool.tile([12, R_pad], F16, name="MnT")
            h2T = pool.tile([13, R_pad], F16, name="h2T")
            nc.gpsimd.memset(h2T[:], 1.0)
            h3T = pool.tile([128, R_pad], F16, name="h3T")
            bns = pool.tile([128, 32], FP, name="bns")
            sqa = pool.tile([128, 32], FP, name="sqa")
            nc.vector.memset(bns[:], 0.0)
            nc.vector.memset(sqa[:], 0.0)
            sqs = pool.tile([128, 512], F16, name="sqs")

            M = pool.tile([128, T, 12], FP, name="Macc")
            S_ = pool.tile([128, T, 4], FP, name="Sacc")
            st = {"norm": 0, "tp": 0, "h2": 0, "fc": 0, "dma": 0}

            def tail_cb(hi):
                # 1. reciprocal + normalize for finished tiles
                ta, tz = st["norm"], hi
                if tz > ta:
                    nc.vector.reciprocal(
                        rn[:, ta:tz, :].rearrange("p t h -> p (t h)"),
                        S_[:, ta:tz, :].rearrange("p t h -> p (t h)"))
                    for hf in range(12):
                        h = hf // 3
                        nc.vector.tensor_tensor(
                            out=Mn[:, ta:tz, hf:hf + 1].rearrange(
                                "p t c -> p (t c)"),
                            in0=M[:, ta:tz, hf:hf + 1].rearrange(
                                "p t c -> p (t c)"),
                            in1=rn[:, ta:tz, h:h + 1].rearrange(
                                "p t c -> p (t c)"), op=OP.mult)
                    st["norm"] = tz
                # 2. transposes in 4-tile batches
                while st["tp"] + 4 <= st["norm"] or \
                        (st["norm"] >= T and st["tp"] < T):
                    g0 = st["tp"]
                    g1_ = min(g0 + 4, T)
                    ps = tps.tile([12, 512], F16, tag="tp")
                    for j, t in enumerate(range(g0, g1_)):
                        nc.tensor.transpose(
                            out=ps[:, j * 128:(j + 1) * 128],
                            in_=Mn[:, t, :], identity=ident[:])
                    nc.scalar.activation(
                        out=MnT[0:12, g0 * 128:g1_ * 128],
                        in_=ps[:, 0:(g1_ - g0) * 128], func=AF.Copy)
                    st["tp"] = g1_
                # 3. h2 matmul in 512-col chunks
                tcols = st["tp"] * 128
                while st["h2"] + 512 <= tcols or \
                        (st["tp"] >= T and st["h2"] < R_pad):
                    j0 = st["h2"]
                    j1 = min(j0 + 512, tcols)
                    ps2 = mms.tile([12, j1 - j0], FP, tag="h2m")
                    nc.tensor.matmul(ps2[:], lhsT=w2b_s[:],
                                     rhs=MnT[:, j0:j1], start=True,
                                     stop=True)
                    nc.scalar.activation(out=h2T[0:12, j0:j1], in_=ps2[:],
                                         func=AF.Relu, bias=b2_s[:, 0:1],
                                         scale=1.0)
                    st["h2"] = j1
                if st["h2"] >= R_pad and R_pad > R and not st.get("zr"):
                    nc.gpsimd.memset(h2T[:, R:R_pad], 0.0)
                    st["zr"] = True
                # 4. fc in 512-col chunks; the chunk containing the dummy
                # columns waits until they are zeroed
                lastchunk = (R // 512) * 512
                fcmax = min(st["h2"], lastchunk) if not st.get("zr") \
                    else (st["h2"] if st["h2"] < R_pad else R_pad)
                while st["fc"] + 512 <= fcmax or \
                        (st["h2"] >= R_pad and st["fc"] < R_pad):
                    j0 = st["fc"]
                    j1 = min(j0 + 512, R_pad)
                    psf = fps.tile([128, j1 - j0], FP, tag="fc")
                    nc.tensor.matmul(psf[:], lhsT=fcw[:], rhs=h2T[:, j0:j1],
                                     start=True, stop=True)
                    i = j0 // 512
                    nc.scalar.activation(out=h3T[:, j0:j1], in_=psf[:],
                                         func=AF.Relu,
                                         accum_out=bns[:, i:i + 1])
                    nc.scalar.activation(out=sqs[:, 0:j1 - j0],
                                         in_=h3T[:, j0:j1], func=AF.Square,
                                         accum_out=sqa[:, i:i + 1])
                    st["fc"] = j1
                # 5. h3 write-out in 2048-col pieces
                while st["dma"] + 2048 <= st["fc"] or \
                        (st["fc"] >= R_pad and st["dma"] < R_pad):
                    j0 = st["dma"]
                    j1 = min(j0 + 2048, R_pad)
                    nc.sync.dma_start(out=h3o.ap()[:, j0:j1],
                                      in_=h3T[:, j0:j1])
                    st["dma"] = j1

            _edge_phase(nc, tc, pool, meta, g2, ed, 3, 4,
                        pool_prod=POOL_PROD_L2, chunk_done_cb=tail_cb,
                        M=M, S_=S_, tb_pool=TB_POOL)

            st2 = pool.tile([128, 2], FP, name="st2")
            nc.vector.tensor_reduce(out=st2[:, 0:1], in_=bns[:],
                                    axis=mybir.AxisListType.X, op=OP.add)
            nc.vector.tensor_reduce(out=st2[:, 1:2], in_=sqa[:],
                                    axis=mybir.AxisListType.X, op=OP.add)
            nc.sync.dma_start(out=sto.ap(), in_=st2[:])
    nc.compile()
    return nc


def _build_l3(meta, n):
    nc = bacc.Bacc("TRN2", target_bir_lowering=False, debug=False,
                   num_devices=N_CORES)
    R_pad = meta["R_pad"]
    h3i = nc.dram_tensor("h3", [128, R_pad], F16, kind="ExternalInput")
    sti = nc.dram_tensor("bnstats", [128, 16], FP, kind="ExternalInput")
    bng = nc.dram_tensor("bng", [128, 1], FP, kind="ExternalInput")
    bnb = nc.dram_tensor("bnb", [128, 1], FP, kind="ExternalInput")
    l2wa = nc.dram_tensor("l2wa", [128, 64], F16, kind="ExternalInput")
    l2bb = nc.dram_tensor("l2bb", [64, 1], FP, kind="ExternalInput")
    owa = nc.dram_tensor("owa", [65, 6], F16, kind="ExternalInput")
    outo = nc.dram_tensor("out", [6, R_pad], FP, kind="ExternalOutput")

    with tile.TileContext(nc) as tc:
        with tc.tile_pool(name="p", bufs=1) as pool, \
             tc.tile_pool(name="ps", bufs=4, space="PSUM") as pp:
            # preload the Sqrt activation table during the DMA wait
            dum = pool.tile([128, 1], FP, name="dum")
            nc.vector.memset(dum[:], 1.0)
            nc.scalar.activation(out=dum[:], in_=dum[:], func=AF.Sqrt)
            sts = _ld(nc, pool, sti, [128, 16], "sts")
            h3s = pool.tile([128, R_pad], F16, name="h3s")
            q = R_pad // 8
            for j in range(0, R_pad, q):
                nc.scalar.dma_start(out=h3s[:, j:j + q],
                                    in_=h3i.ap()[:, j:j + q])
            bng_s = _ld(nc, pool, bng, [128, 1], "bng_s")
            bnb_s = _ld(nc, pool, bnb, [128, 1], "bnb_s")
            l2w = _ld(nc, pool, l2wa, [128, 64], "l2w", dt=F16)
            l2b = _ld(nc, pool, l2bb, [64, 1], "l2b")
            ow = _ld(nc, pool, owa, [65, 6], "ow", dt=F16)

            red = pool.tile([128, 2], FP, name="red")
            nc.vector.tensor_reduce(
                out=red[:], in_=sts[:].rearrange("p (s c) -> p s c", s=2),
                axis=mybir.AxisListType.X, op=OP.add)
            mu = pool.tile([128, 1], FP, name="mu")
            nc.vector.tensor_scalar(out=mu[:], in0=red[:, 0:1],
                                    scalar1=1.0 / n, scalar2=None,
                                    op0=OP.mult)
            m2 = pool.tile([128, 1], FP, name="m2")
            nc.vector.tensor_scalar(out=m2[:], in0=red[:, 1:2],
                                    scalar1=1.0 / n, scalar2=None,
                                    op0=OP.mult)
            var = pool.tile([128, 1], FP, name="var")
            nc.vector.tensor_tensor(out=var[:], in0=mu[:], in1=mu[:],
                                    op=OP.mult)
            nc.vector.tensor_tensor(out=var[:], in0=m2[:], in1=var[:],
                                    op=OP.subtract)
            epsb = pool.tile([128, 1], FP, name="epsb")
            nc.vector.memset(epsb[:], BN_EPS)
            sd = pool.tile([128, 1], FP, name="sd")
            nc.scalar.activation(out=sd[:], in_=var[:], func=AF.Sqrt,
                                 bias=epsb[:], scale=1.0)
            rsig = pool.tile([128, 1], FP, name="rsig")
            nc.vector.reciprocal(rsig[:], sd[:])
            scale = pool.tile([128, 1], FP, name="scale")
            nc.vector.tensor_tensor(out=scale[:], in0=bng_s[:], in1=rsig[:],
                                    op=OP.mult)
            shift = pool.tile([128, 1], FP, name="shift")
            nc.vector.tensor_tensor(out=shift[:], in0=mu[:], in1=scale[:],
                                    op=OP.mult)
            nc.vector.tensor_tensor(out=shift[:], in0=bnb_s[:], in1=shift[:],
                                    op=OP.subtract)

            hbn = pool.tile([128, R_pad], F16, name="hbn")
            h4a = pool.tile([65, R_pad], F16, name="h4a")
            nc.gpsimd.memset(h4a[:], 1.0)
            outT = pool.tile([6, R_pad], FP, name="outT")
            chunks = [(j, min(j + 512, R_pad)) for j in range(0, R_pad, 512)]
            for (j0, j1) in chunks:
                nc.vector.tensor_scalar(out=hbn[:, j0:j1], in0=h3s[:, j0:j1],
                                        scalar1=scale[:], scalar2=shift[:],
                                        op0=OP.mult, op1=OP.add)
                ps = pp.tile([64, j1 - j0], FP, tag="l2")
                nc.tensor.matmul(ps[:], lhsT=l2w[:], rhs=hbn[:, j0:j1],
                                 start=True, stop=True)
                nc.vector.tensor_scalar(out=h4a[0:64, j0:j1], in0=ps[:],
                                        scalar1=l2b[:, 0:1], scalar2=None,
                                        op0=OP.add)
                ps2 = pp.tile([6, j1 - j0], FP, tag="out")
                nc.tensor.matmul(ps2[:], lhsT=ow[:], rhs=h4a[:, j0:j1],
                                 start=True, stop=True)
                nc.scalar.activation(out=outT[:, j0:j1], in_=ps2[:],
                                     func=AF.Sigmoid)
            nc.sync.dma_start(out=outo.ap(), in_=outT[:])
    nc.compile()
    return nc


# --------------------------------------------------------------------------
# Orchestration
# --------------------------------------------------------------------------

def _bcast(a, cols):
    return np.ascontiguousarray(np.broadcast_to(
        np.asarray(a, np.float32).reshape(1, -1), (128, cols)))


def _run(nc, in_maps):
    import time as _t
    t0 = _t.perf_counter()
    res = run_bass_kernel_spmd(nc, in_maps, list(range(N_CORES)))
    LAUNCH_WALL.append(_t.perf_counter() - t0)
    LAST_RESULTS.append(res)
    return res.results


def _rows_to_pernode(meta, arrs):
    R = meta["R"]
    F = arrs[0].shape[1]
    out = np.empty((R * N_CORES, F), arrs[0].dtype)
    for c in range(N_CORES):
        out[meta["rows_node"][c]] = arrs[c][:R]
    return out


def EXTRA_TSIM_BUILDERS(meta, n):
    return {
        "l0": lambda: _build_l0(meta),
        "l1": lambda: _build_l1(meta),
        "l2": lambda: _build_l2(meta, n),
        "l3": lambda: _build_l3(meta, n),
    }


def kernel(x, edge_index, W1, a_src1, a_dst1, b1, W2, a_src2, a_dst2, b2,
           fc_W, fc_b, bn_g, bn_b, l2_W, l2_b, out_W, out_b):
    global LAST_RESULTS
    LAST_RESULTS = []
    x = np.asarray(x, np.float32)
    n = x.shape[0]
    ekey = (n, np.asarray(edge_index).shape[1])
    meta = _PROG_CACHE.get(("meta", ekey))
    fp = np.asarray(edge_index)[:, :: max(1, ekey[1] // 64)]
    if meta is None or not np.array_equal(meta["_fp"], fp):
        meta = _preprocess(np.asarray(edge_index), n)
        meta["_fp"] = fp.copy()
        _PROG_CACHE.clear()
        _PROG_CACHE[("meta", ekey)] = meta
    R, R_pad = meta["R"], meta["R_pad"]
    if ("l0", ekey) not in _PROG_CACHE:
        _PROG_CACHE[("l0", ekey)] = _build_l0(meta)
        _PROG_CACHE[("l1", ekey)] = _build_l1(meta)
        _PROG_CACHE[("l2", ekey)] = _build_l2(meta, n)
        _PROG_CACHE[("l3", ekey)] = _build_l3(meta, n)

    # ---- launch 0: per-node u / es1 / ed1
    T = meta["T"]

    def to_dev(a):      # [R_pad, C] rank-major -> [128, T*C]
        C = a.shape[1]
        return np.ascontiguousarray(
            a.reshape(T, 128, C).transpose(1, 0, 2).reshape(128, T * C))

    def from_dev(a, C):  # [128, T*C] -> [R_pad, C] rank-major
        return a.reshape(128, T, C).transpose(1, 0, 2).reshape(R_pad, C)

    tail15 = np.concatenate([
        np.asarray(W1, np.float32).reshape(-1),
        np.asarray(a_src1, np.float32).reshape(-1),
        np.asarray(a_dst1, np.float32).reshape(-1)]).reshape(1, 15)
    in_maps = []
    for c in range(N_CORES):
        xr = np.zeros((R_pad, 3), np.float32)
        xr[:R] = x[meta["rows_node"][c]]
        # c-major planes [x0(T) x1(T) x2(T)] + [W1|a_src1|a_dst1]
        xrd = xr.reshape(T, 128, 3).transpose(1, 2, 0).reshape(128, 3 * T)
        in_maps.append(dict(
            xr=np.ascontiguousarray(np.concatenate(
                [xrd, np.broadcast_to(tail15, (128, 15))], axis=1))))
    r0 = _run(_PROG_CACHE[("l0", ekey)], in_maps)

    def u_unpack(a):   # [128, 4T] c-major -> [R_pad, 4] rank-major
        return a.reshape(128, 4, T).transpose(2, 0, 1).reshape(R_pad, 4)

    u4 = [u_unpack(r0[c]["u"]) for c in range(N_CORES)]
    u_pn = _rows_to_pernode(meta, [a[:, 0:3] for a in u4])
    es1_pn = _rows_to_pernode(meta, [a[:, 3:4] for a in u4])

    # ---- launch 1
    g1 = _make_stream(meta, u_pn, es1_pn)
    in_maps = []
    for c in range(N_CORES):
        in_maps.append(dict(
            g1=g1[c], ed1=np.ascontiguousarray(r0[c]["ed1"]),
            b1b=_bcast(b1, 3), w2b=_bcast(W2, 36), as2=_bcast(a_src2, 12),
            ad2=_bcast(a_dst2, 12)))
    r1 = _run(_PROG_CACHE[("l1", ekey)], in_maps)

    x2_pn = _rows_to_pernode(meta, [from_dev(r1[c]["x2"], 3)
                                    for c in range(N_CORES)])
    es4_pn = _rows_to_pernode(meta, [from_dev(r1[c]["es4"], 4)
                                    for c in range(N_CORES)])

    # ---- launch 2 (GAT2 + fc + BN partial stats)
    g2 = _make_stream(meta, x2_pn, es4_pn)
    fcwa = np.vstack([np.asarray(fc_W, np.float32),
                      np.asarray(fc_b, np.float32)[None, :]]).astype(
        np.float16)
    # block-diagonal per-head W2: w2blk[(h,f), hc] = W2[f, hc] iff hc//3 == h
    W2f = np.asarray(W2, np.float32)
    w2blk = np.zeros((12, 12), np.float32)
    for h in range(4):
        w2blk[h * 3:(h + 1) * 3, h * 3:(h + 1) * 3] = W2f[:, h * 3:(h + 1) * 3]
    w2blk = w2blk.astype(np.float16)
    in_maps = []
    for c in range(N_CORES):
        in_maps.append(dict(
            g2=g2[c], ed4=np.ascontiguousarray(r1[c]["ed4"]),
            w2blk=w2blk, b2c=np.asarray(b2, np.float32).reshape(12, 1),
            fcwa=fcwa))
    r2 = _run(_PROG_CACHE[("l2", ekey)], in_maps)

    stats = np.zeros((128, 16), np.float32)
    for c in range(N_CORES):
        stats[:, c] = r2[c]["bnstat"][:, 0]
        stats[:, 8 + c] = r2[c]["bnstat"][:, 1]

    # ---- launch 3 (BN finalize/apply + MLP output)
    l2wa = np.asarray(l2_W, np.float32).astype(np.float16)
    owa = np.vstack([np.asarray(out_W, np.float32),
                     np.asarray(out_b, np.float32)[None, :]]).astype(
        np.float16)
    in_maps = []
    for c in range(N_CORES):
        in_maps.append(dict(
            h3=r2[c]["h3"], bnstats=stats,
            bng=np.asarray(bn_g, np.float32).reshape(128, 1),
            bnb=np.asarray(bn_b, np.float32).reshape(128, 1),
            l2wa=l2wa,
            l2bb=np.asarray(l2_b, np.float32).reshape(64, 1),
            owa=owa))
    r3 = _run(_PROG_CACHE[("l3", ekey)], in_maps)

    out = np.zeros((n, 6), np.float32)
    for c in range(N_CORES):
        out[meta["rows_node"][c]] = r3[c]["out"][:, :R].T
    return out
